# revision 24
# baseline (speedup 1.0000x reference)
"""Trainium2 Bass kernel for nn_BaseDTA (quadrant dual-token attention).

Data-parallel over batch: each of the 8 NeuronCores processes one sample
end-to-end (4 quadrant MHSA sequences of length 1026 + gating + a second
4096x256 cross-attention). No collectives.

v2: fp8e4 DoubleRow everywhere the math allows.
 - Q/K live in a "DR-pair" layout [128p, 2s, T]: head h = 4g+j owns
   partitions 32j..32j+16 of tile-group g; slot s holds channels 16s+k.
   Scores for one head are a single DR matmul (contraction 16x2=32).
 - V lives zero-padded per (g, j): V8Z[:, g, j, slot, 128ch] has head j's
   32 channels at cols 32j..32j+32, zeros elsewhere, so 4 heads accumulate
   into one full-128-row PSUM bank (DR col-packed outputs are rejected by
   walrus, zero-padding is free under the cost model).
 - Keys tile into 10 slots of 128 (slot 8 holds the 2 lt/gt keys, slot 9 is
   zero padding) -> 5 DoubleRow key-groups of 256 for AV/denominator.
 - Softmax denominators via ONESZ (ones at head j's columns) DR matmuls.
 - exp runs on ACT out of a 6-bank PSUM ring straight into fp8 At tiles.
 - attn2 K/V tensors are scaled x64 before fp8 (they sit near e4m3's
   subnormal range); the scale is folded into exp(scale=) and the output
   normalization.
"""

import math
from collections import deque

import numpy as np
import ml_dtypes

import concourse.bass as bass
import concourse.mybir as mybir
import concourse.tile as tile
from concourse import bacc
from concourse.bass_utils import run_bass_kernel_spmd

F32 = mybir.dt.float32
FP8 = mybir.dt.float8e4
AF = mybir.ActivationFunctionType
AX = mybir.AxisListType
ALU = mybir.AluOpType
DR = mybir.MatmulPerfMode.DoubleRow

B, C, H, W = 8, 256, 64, 64
h2, w2 = H // 2, W // 2          # 32
NH = 8
HD = C // NH                     # 32
HW = H * W                       # 4096
PIX = h2 * w2                    # 1024
T = PIX + 2                      # 1026
QC = 342                         # query chunk (3 per T)
NKT = 9                          # real key tiles (8x128 + 2)
NSLOT = 10                       # key slots incl zero pad
NKG = 5                          # DR key groups of 256
RING = 6                         # sp ring banks
K2SC = 64.0                      # attn2 K scaling before fp8


def _build():
    nc = bacc.Bacc(trn_type="TRN2", target_bir_lowering=False, num_devices=8)

    x_d = nc.dram_tensor("x", [C, HW], F32, kind="ExternalInput")
    wq8_d = nc.dram_tensor("wq8", [128, 2 * 2 * 2 * 128], FP8, kind="ExternalInput")
    wk8_d = nc.dram_tensor("wk8", [128, 2 * 2 * 2 * 128], FP8, kind="ExternalInput")
    wv8_d = nc.dram_tensor("wv8", [128, 2 * C], FP8, kind="ExternalInput")
    wo8_d = nc.dram_tensor("wo8", [128, 2 * C], FP8, kind="ExternalInput")
    wf8l_d = nc.dram_tensor("wf8l", [128, 2 * C], FP8, kind="ExternalInput")
    wf8r_d = nc.dram_tensor("wf8r", [128, 2 * C], FP8, kind="ExternalInput")
    bqp_d = nc.dram_tensor("bqp", [128, 4], F32, kind="ExternalInput")
    bkp_d = nc.dram_tensor("bkp", [128, 4], F32, kind="ExternalInput")
    bvr_d = nc.dram_tensor("bvr", [128, C], F32, kind="ExternalInput")
    bo_d = nc.dram_tensor("bo", [C, 1], F32, kind="ExternalInput")
    bfp64_d = nc.dram_tensor("bfp64", [128, 2], F32, kind="ExternalInput")
    bfr64_d = nc.dram_tensor("bfr64", [128, C], F32, kind="ExternalInput")
    g_names = ["wrow_rep", "brow_rep", "wcol_rep", "bcol_rep"]
    g_d = {n: nc.dram_tensor(n, [128, h2], F32, kind="ExternalInput") for n in g_names}
    wgt_rep_d = nc.dram_tensor("wgt_rep", [128, H], F32, kind="ExternalInput")
    bgt_rep_d = nc.dram_tensor("bgt_rep", [128, H], F32, kind="ExternalInput")
    o_d = nc.dram_tensor("o", [C, HW], F32, kind="ExternalOutput")

    with tile.TileContext(nc) as tc:
        _emit(nc, tc, x_d, wq8_d, wk8_d, wv8_d, wo8_d, wf8l_d, wf8r_d, bqp_d, bkp_d,
              bvr_d, bo_d, bfp64_d, bfr64_d, g_d, wgt_rep_d, bgt_rep_d, o_d)
    nc.compile()
    return nc


def _emit(nc, tc, x_d, wq8_d, wk8_d, wv8_d, wo8_d, wf8l_d, wf8r_d, bqp_d, bkp_d,
          bvr_d, bo_d, bfp64_d, bfr64_d, g_d, wgt_rep_d, bgt_rep_d, o_d):
    with tc.tile_pool(name="singles", bufs=1) as sg:
        X = sg.tile([128, 2, HW], F32)
        FW = sg.tile([128, 2, HW], F32)
        RP = sg.tile([128, 2, 4 * H], F32)
        WQ8f = sg.tile([128, 2 * 2 * 2 * 128], FP8)
        WK8f = sg.tile([128, 2 * 2 * 2 * 128], FP8)
        WV8f = sg.tile([128, 2 * C], FP8)
        WO8f = sg.tile([128, 2 * C], FP8)
        WF8Lf = sg.tile([128, 2 * C], FP8)
        WF8Rf = sg.tile([128, 2 * C], FP8)
        # all lhsT layouts keep the ct slot-pair contiguous (dual-fp8
        # ldweights requires slot stride == inner size)
        WQ8 = WQ8f.rearrange("p (g s ct d) -> p g s ct d", g=2, s=2, ct=2)
        WK8 = WK8f.rearrange("p (g s ct d) -> p g s ct d", g=2, s=2, ct=2)
        WV8 = WV8f.rearrange("p (ct d) -> p ct d", ct=2)
        WO8 = WO8f.rearrange("p (mt ct d) -> p mt ct d", mt=2, ct=2)
        WF8L = WF8Lf.rearrange("p (s ct d) -> p s ct d", s=2, ct=2)
        WF8R = WF8Rf.rearrange("p (ct d) -> p ct d", ct=2)
        BQP = sg.tile([128, 4], F32)
        BKP = sg.tile([128, 4], F32)
        BVr = sg.tile([128, C], F32)
        BO = sg.tile([128, 2, 1], F32)
        BFP64 = sg.tile([128, 2], F32)
        BFr64 = sg.tile([128, C], F32)
        WRr = sg.tile([128, h2], F32)
        BRr = sg.tile([128, h2], F32)
        WCr = sg.tile([128, h2], F32)
        BCr = sg.tile([128, h2], F32)
        WGr = sg.tile([128, H], F32)
        BGr = sg.tile([128, H], F32)
        GT = sg.tile([128, 2, 1], F32)
        ONESZ = sg.tile([128, 4, 2, 128], FP8)    # dp lhsT, m<4
        ONESZ4 = sg.tile([128, 4, 2, 128], FP8)   # dp lhsT, m=4 (2 real keys)
        ONES2 = sg.tile([128, 2, 128], FP8)       # attn2 d2 lhsT

        for ct in range(2):
            for xc in range(4):
                nc.sync.dma_start(
                    out=X[:, ct, xc * 1024:(xc + 1) * 1024],
                    in_=x_d[ct * 128:(ct + 1) * 128, xc * 1024:(xc + 1) * 1024])
        nc.sync.dma_start(out=WQ8f[:, :], in_=wq8_d[:, :])
        nc.sync.dma_start(out=WK8f[:, :], in_=wk8_d[:, :])
        nc.sync.dma_start(out=WV8f[:, :], in_=wv8_d[:, :])
        nc.sync.dma_start(out=WO8f[:, :], in_=wo8_d[:, :])
        nc.sync.dma_start(out=WF8Lf[:, :], in_=wf8l_d[:, :])
        nc.sync.dma_start(out=WF8Rf[:, :], in_=wf8r_d[:, :])
        nc.sync.dma_start(out=BQP[:, :], in_=bqp_d[:, :])
        nc.sync.dma_start(out=BKP[:, :], in_=bkp_d[:, :])
        nc.sync.dma_start(out=BVr[:, :], in_=bvr_d[:, :])
        for mt in range(2):
            nc.sync.dma_start(out=BO[:, mt, :],
                              in_=bo_d[mt * 128:(mt + 1) * 128, :])
        nc.sync.dma_start(out=BFP64[:, :], in_=bfp64_d[:, :])
        nc.sync.dma_start(out=BFr64[:, :], in_=bfr64_d[:, :])
        for n, dst in [("wrow_rep", WRr), ("brow_rep", BRr),
                       ("wcol_rep", WCr), ("bcol_rep", BCr)]:
            nc.sync.dma_start(out=dst[:, :], in_=g_d[n][:, :])
        nc.sync.dma_start(out=WGr[:, :], in_=wgt_rep_d[:, :])
        nc.sync.dma_start(out=BGr[:, :], in_=bgt_rep_d[:, :])

        nc.vector.memset(ONESZ[:, :, :, :], 0.0)
        nc.vector.memset(ONESZ4[:, :, :, :], 0.0)
        nc.vector.memset(ONES2[:, :, :], 1.0)
        for j in range(4):
            nc.vector.memset(ONESZ[:, j, :, 32 * j:32 * j + 32], 1.0)
            nc.vector.memset(ONESZ4[0:2, j, 0, 32 * j:32 * j + 32], 1.0)

        # global token: mean over all pixels
        for ct in range(2):
            nc.vector.reduce_sum(GT[:, ct, :], X[:, ct, :], AX.X)
            nc.vector.tensor_scalar_mul(GT[:, ct, :], GT[:, ct, :], 1.0 / HW)

        with (
            tc.tile_pool(name="ypool", bufs=2) as ypool,
            tc.tile_pool(name="y8pool", bufs=1) as y8pool,
            tc.tile_pool(name="qkpool", bufs=2) as qkpool,
            tc.tile_pool(name="v8zpool", bufs=2) as v8zpool,
            tc.tile_pool(name="atpool", bufs=2) as atpool,
            tc.tile_pool(name="aopool", bufs=1) as aopool,
            tc.tile_pool(name="scratch", bufs=2) as scratch,
            tc.tile_pool(name="gpool", bufs=1) as gpool,
            tc.tile_pool(name="ps_ring", bufs=1, space="PSUM") as ps_ring,
            tc.tile_pool(name="ps_acc", bufs=2, space="PSUM") as ps_acc,
        ):
            # pre-zero the At buffers' pad slots (8, 9) once per physical buf
            for _ in range(2):
                At = atpool.tile([128, NSLOT, NH, QC], FP8, tag="At", name="At")
                nc.vector.memset(At[:, 8:10, :, :], 0.0)
            v8z_bufs = []
            for _ in range(2):
                V8Z = v8zpool.tile([128, 2, 4, NSLOT, 128], FP8, tag="V8Z",
                                   name="V8Z")
                nc.vector.memset(V8Z[:, :, :, :, :], 0.0)
                v8z_bufs.append(V8Z)

            quad_state = {}
            # Shared FIFO of deferred emissions. Every user of the shared
            # "acc" PSUM pool allocates its tile INSIDE its closure, so
            # allocation order == emission order and the 2-slot rotation
            # stays correctly nested.
            backlog = deque()

            def flush(k):
                for _ in range(min(k, len(backlog))):
                    backlog.popleft()()

            def acc_tile(name):
                return ps_acc.tile([128, 512], F32, tag="acc", name=name)

            def prep(q):
                """Emit Y/Y8 and queue Q/K/V projections for quad q."""
                Y = ypool.tile([128, 2, T], F32, tag="Y", name="Y")
                Y8 = y8pool.tile([128, 2, T], FP8, tag="Y8", name="Y8")
                # token-blocked fp8 Y for the V-proj lhsT (slot pair = ct must
                # be contiguous); slot 8 holds lt/gt + zero pad
                Y8V = y8pool.tile([128, NKT, 2, 128], FP8, tag="Y8V",
                                  name="Y8V")
                QDR = qkpool.tile([128, 2, 2, T], FP8, tag="QDR", name="QDR")
                # key-blocked K: [p, g, kt, ct-slot, 128]; kt=8 cols 2..128
                # are zero-padded fake keys (their exp=1 rows are masked by
                # V8Z/ONESZ4 zeros)
                KDR = qkpool.tile([128, 2, NKT, 2, 128], FP8, tag="KDR",
                                  name="KDR")
                V8Z = v8zpool.tile([128, 2, 4, NSLOT, 128], FP8, tag="V8Z",
                                   name="V8Zq")
                LTG = scratch.tile([128, 2, 2], F32, tag="LTG", name="LTG")
                quad_state[q] = dict(Y=Y, QDR=QDR, KDR=KDR, V8Z=V8Z, LTG=LTG)
                nc.vector.memset(Y8V[:, 8, :, 2:128], 0.0)
                nc.vector.memset(KDR[:, :, 8, :, 2:128], 0.0)
                r0, c0 = h2 * (q // 2), w2 * (q % 2)
                for ct in range(2):
                    xv = X[:, ct, :].rearrange("p (a b) -> p a b", a=H)[
                        :, r0:r0 + h2, c0:c0 + w2]
                    yq = Y[:, ct, 0:PIX].rearrange("p (a b) -> p a b", a=h2)
                    nc.vector.tensor_copy(yq, xv)
                    nc.vector.reduce_sum(LTG[:, ct, 0:1], Y[:, ct, 0:PIX], AX.X)
                    nc.vector.tensor_scalar_mul(LTG[:, ct, 0:1],
                                                LTG[:, ct, 0:1], 1.0 / PIX)
                    nc.vector.tensor_copy(LTG[:, ct, 1:2], GT[:, ct, :])
                    nc.vector.tensor_copy(Y[:, ct, PIX:T], LTG[:, ct, :])
                    nc.vector.tensor_copy(Y8[:, ct, :], Y[:, ct, :])
                    nc.vector.tensor_copy(
                        Y8V[:, 0:8, ct, :],
                        Y[:, ct, 0:PIX].rearrange("p (a b) -> p a b", b=128))
                    nc.vector.tensor_copy(Y8V[:, 8, ct, 0:2],
                                          Y[:, ct, PIX:T])
                    yield
                # Q projection (flat layout, rhs-only -> strided slots fine)
                for g in range(2):
                    for s in range(2):
                        for qs in range(0, T, QC):
                            def qp(g=g, s=s, qs=qs, Y8=Y8, QDR=QDR):
                                pq = acc_tile("pq")
                                nc.tensor.matmul(
                                    pq[:, 0:QC], WQ8[:, g, s, :, :],
                                    Y8[:, :, qs:qs + QC],
                                    start=True, stop=True, perf_mode=DR)
                                nc.vector.tensor_scalar_add(
                                    QDR[:, g, s, qs:qs + QC], pq[:, 0:QC],
                                    BQP[:, 2 * g + s:2 * g + s + 1])
                            backlog.append(qp)
                            yield
                # K projection into the key-blocked layout (256-token chunks)
                for g in range(2):
                    for s in range(2):
                        for c4 in range(4):
                            def kp(g=g, s=s, c4=c4, Y8=Y8, KDR=KDR):
                                pq = acc_tile("pk")
                                nc.tensor.matmul(
                                    pq[:, 0:256], WK8[:, g, s, :, :],
                                    Y8[:, :, 256 * c4:256 * c4 + 256],
                                    start=True, stop=True, perf_mode=DR)
                                nc.vector.tensor_scalar_add(
                                    KDR[:, g, 2 * c4:2 * c4 + 2, s, :],
                                    pq[:, 0:256].rearrange(
                                        "p (a b) -> p a b", b=128),
                                    BKP[:, 2 * g + s:2 * g + s + 1])
                            backlog.append(kp)
                            yield
                        def kp_tail(g=g, s=s, Y8=Y8, KDR=KDR):
                            pq = acc_tile("pkt")
                            nc.tensor.matmul(
                                pq[:, 0:2], WK8[:, g, s, :, :],
                                Y8[:, :, 1024:1026],
                                start=True, stop=True, perf_mode=DR)
                            nc.vector.tensor_scalar_add(
                                KDR[:, g, 8, s, 0:2], pq[:, 0:2],
                                BKP[:, 2 * g + s:2 * g + s + 1])
                        backlog.append(kp_tail)
                        yield
                for tt in range(NKT):
                    def vp(tt=tt, Y8V=Y8V, V8Z=V8Z):
                        n = 128 if tt < 8 else T - 8 * 128
                        pv = acc_tile("pv")
                        nc.tensor.matmul(pv[:, 0:C], Y8V[:, tt, :, :],
                                         WV8[:, :, :],
                                         start=True, stop=True, perf_mode=DR)
                        for g in range(2):
                            for j in range(4):
                                cs = 128 * g + 32 * j
                                nc.vector.tensor_add(
                                    V8Z[0:n, g, j, tt, 32 * j:32 * j + 32],
                                    pv[0:n, cs:cs + 32], BVr[0:n, cs:cs + 32])
                    backlog.append(vp)
                    yield

            def att(q):
                """Attention for quad q. Yields after each kt exp so the
                driver can interleave prep(q+1) work."""
                r0, c0 = h2 * (q // 2), w2 * (q % 2)
                st = quad_state[q]
                Y, QDR, KDR, V8Z, LTG = (st["Y"], st["QDR"], st["KDR"],
                                         st["V8Z"], st["LTG"])
                AT = aopool.tile([128, 2, T], FP8, tag="AT", name="AT")
                OT = aopool.tile([128, 2, T], F32, tag="OT", name="OT")

                sp = ps_ring.tile([128, RING, 512], F32, tag="sp", name="sp")
                wcount = 0
                for qs in range(0, T, QC):
                    At = atpool.tile([128, NSLOT, NH, QC], FP8, tag="At",
                                     name="Atq")
                    for kt in range(NKT):
                        # 3 windows of heads (3, 3, 2); windows alternate
                        # between bank triples [0-2] and [3-5] (reuse is
                        # always two windows behind = double buffered)
                        for h0, h1 in ((0, 3), (3, 6), (6, 8)):
                            pos = 3 * (wcount % 2)
                            wcount += 1
                            for i, h in enumerate(range(h0, h1)):
                                g, j = h // 4, h % 4
                                nc.tensor.matmul(
                                    sp[:, pos + i, 0:QC],
                                    KDR[32 * j:32 * j + 16, g, kt, :, :],
                                    QDR[32 * j:32 * j + 16, g, :, qs:qs + QC],
                                    start=True, stop=True, perf_mode=DR,
                                    tile_position=(32 * j, 0))
                            nc.scalar.activation(
                                out=At[:, kt, h0:h1, :],
                                in_=sp[:, pos:pos + (h1 - h0), 0:QC],
                                func=AF.Exp)
                            flush(1)
                            yield

                    for g in range(2):
                        # queue AV + dp + normalize for this (g, qs); the
                        # avp/dp banks are allocated at flush time (m == 0)
                        # to keep the acc-pool rotation FIFO
                        hold = {}

                        def mk_av(m, g=g, At=At, hold=hold, V8Z=V8Z):
                            def go():
                                if m == 0:
                                    hold["avp"] = acc_tile("avp")
                                    hold["dp"] = acc_tile("dp")
                                avp, dp = hold["avp"], hold["dp"]
                                for j in range(4):
                                    nc.tensor.matmul(
                                        avp[:, 0:QC],
                                        V8Z[:, g, j, 2 * m:2 * m + 2, :],
                                        At[:, 2 * m:2 * m + 2, 4 * g + j, :],
                                        start=(m == 0 and j == 0),
                                        stop=(m == 4 and j == 3),
                                        perf_mode=DR)
                                    oz = ONESZ if m < 4 else ONESZ4
                                    nc.tensor.matmul(
                                        dp[:, 0:QC],
                                        oz[:, j, :, :],
                                        At[:, 2 * m:2 * m + 2, 4 * g + j, :],
                                        start=(m == 0 and j == 0),
                                        stop=(m == 4 and j == 3),
                                        perf_mode=DR)
                            return go

                        for m in range(NKG):
                            backlog.append(mk_av(m))

                        def norm(g=g, qs=qs, hold=hold, AT=AT):
                            dr = scratch.tile([128, QC], F32, tag="dr",
                                              name="dr")
                            nc.vector.reciprocal(dr[:, :], hold["dp"][:, 0:QC])
                            nc.vector.tensor_mul(AT[:, g, qs:qs + QC],
                                                 hold["avp"][:, 0:QC],
                                                 dr[:, :])
                        backlog.append(norm)

                        if g == 1:
                            def oproj(qs=qs, AT=AT, OT=OT, Y=Y):
                                for mt in range(2):
                                    po = acc_tile("po")
                                    nc.tensor.matmul(
                                        po[:, 0:QC],
                                        WO8[:, mt, :, :],
                                        AT[:, :, qs:qs + QC],
                                        start=True, stop=True, perf_mode=DR)
                                    nc.vector.scalar_tensor_tensor(
                                        OT[:, mt, qs:qs + QC], po[:, 0:QC],
                                        BO[:, mt, :], Y[:, mt, qs:qs + QC],
                                        op0=ALU.add, op1=ALU.add)
                            backlog.append(oproj)
                while backlog:
                    backlog.popleft()()
                    yield

                # gating
                for ct in range(2):
                    ltp = OT[:, ct, PIX:PIX + 1]
                    gtp = OT[:, ct, PIX + 1:T]
                    row = gpool.tile([128, h2], F32, tag="row", name="row")
                    col = gpool.tile([128, h2], F32, tag="col", name="col")
                    nc.vector.scalar_tensor_tensor(row[:, :], WRr[:, :], ltp,
                                                   BRr[:, :], op0=ALU.mult,
                                                   op1=ALU.add)
                    nc.vector.scalar_tensor_tensor(col[:, :], WCr[:, :], ltp,
                                                   BCr[:, :], op0=ALU.mult,
                                                   op1=ALU.add)
                    prod = gpool.tile([128, h2, w2], F32, tag="prod",
                                      name="prod")
                    nc.vector.tensor_mul(
                        prod[:, :, :],
                        row[:, :, None].broadcast_to([128, h2, w2]),
                        col[:, None, :].broadcast_to([128, h2, w2]))
                    eg = gpool.tile([128, h2, w2], F32, tag="eg", name="eg")
                    nc.scalar.activation(out=eg[:, :, :], in_=prod[:, :, :],
                                         func=AF.Exp)
                    # sigmoid(z) = e/(1+e): stays on the Exp ACT table
                    e1 = gpool.tile([128, h2, w2], F32, tag="prod", name="e1")
                    nc.vector.tensor_scalar_add(e1[:, :, :], eg[:, :, :], 1.0)
                    nc.vector.reciprocal(e1[:, :, :], e1[:, :, :])
                    nc.vector.tensor_mul(eg[:, :, :], eg[:, :, :], e1[:, :, :])
                    fv = FW[:, ct, :].rearrange("p (a b) -> p a b", a=H)[
                        :, r0:r0 + h2, c0:c0 + w2]
                    xp = OT[:, ct, 0:PIX].rearrange("p (a b) -> p a b", a=h2)
                    nc.vector.tensor_mul(fv, xp, eg[:, :, :])
                    rp_v = RP[:, ct, :].rearrange("p (a b) -> p a b", b=4)[
                        :, :, q]
                    nc.vector.scalar_tensor_tensor(rp_v, WGr[:, :], gtp,
                                                   BGr[:, :], op0=ALU.mult,
                                                   op1=ALU.add)
                    yield

            def drain(gen):
                if gen is None:
                    return
                for _ in gen:
                    pass

            drain(prep(0))
            # prep(0)'s projection closures must be emitted before att(0)
            # emits scores that read QDR/KDR (no writer yet = no dependency)
            while backlog:
                backlog.popleft()()
            for q in range(4):
                a = att(q)
                p = prep(q + 1) if q < 3 else None
                while True:
                    try:
                        next(a)
                    except StopIteration:
                        break
                    if p is not None:
                        for _ in range(2):
                            try:
                                next(p)
                            except StopIteration:
                                p = None
                                break
                drain(p)

        _emit_attn2(nc, tc, FW, RP, WF8L, WF8R, BFP64, BFr64, ONES2, o_d)


def _emit_attn2(nc, tc, FW, RP, WF8L, WF8R, BFP64, BFr64, ONES2, o_d):
    with (
        tc.tile_pool(name="a2pool", bufs=1) as a2pool,
        tc.tile_pool(name="opool", bufs=2) as opool,
        tc.tile_pool(name="ps2_s", bufs=2, space="PSUM") as ps2_s,
        tc.tile_pool(name="ps2_acc", bufs=2, space="PSUM") as ps2_acc,
    ):
        FW8 = a2pool.tile([128, 2, HW], FP8, tag="FW8", name="FW8")
        RP8R = a2pool.tile([128, 2, 4 * H], FP8, tag="RP8R", name="RP8R")
        RP8L = a2pool.tile([128, 2, 2, 128], FP8, tag="RP8L", name="RP8L")
        K2T8 = a2pool.tile([128, 2, 2, 128], FP8, tag="K2T8", name="K2T8")
        K2K8 = a2pool.tile([128, 2, 2, 128], FP8, tag="K2K8", name="K2K8")
        A28 = a2pool.tile([128, 2, HW], FP8, tag="A28", name="A28")

        for ct in range(2):
            nc.vector.tensor_copy(FW8[:, ct, :], FW[:, ct, :])
            nc.vector.tensor_scalar_mul(RP8R[:, ct, :], RP[:, ct, :], K2SC)
            nc.vector.tensor_scalar_mul(
                RP8L[:, :, ct, :],
                RP[:, ct, :].rearrange("p (a b) -> p a b", b=128), K2SC)

        # K2T8[p, kt, s, key] = 64*K2T[cout=128s+p, key=128kt+key]
        # K2K8[p, mt, s, c] = 64*K2K[key=128s+p, cout=128mt+c]
        for s in range(2):
            pk = ps2_acc.tile([128, 512], F32, tag="acc2", name="pk")
            nc.tensor.matmul(pk[:, 0:C], WF8L[:, s, :, :],
                             RP8R[:, :, :], start=True, stop=True,
                             perf_mode=DR)
            nc.vector.tensor_scalar_add(
                K2T8[:, :, s, :],
                pk[:, 0:C].rearrange("p (a b) -> p a b", b=128),
                BFP64[:, s:s + 1])
        for s in range(2):
            pk = ps2_acc.tile([128, 512], F32, tag="acc2", name="pk2")
            nc.tensor.matmul(pk[:, 0:C], RP8L[:, s, :, :],
                             WF8R[:, :, :], start=True, stop=True,
                             perf_mode=DR)
            nc.vector.tensor_add(
                K2K8[:, :, s, :],
                pk[:, 0:C].rearrange("p (a b) -> p a b", b=128),
                BFr64[:, :].rearrange("p (a b) -> p a b", b=128))

        a2scale = 1.0 / (math.sqrt(C) * K2SC)

        def a2_scores(qb):
            s2 = ps2_s.tile([128, 2, 512], F32, tag="s2", name="s2")
            for kt in range(2):
                nc.tensor.matmul(s2[:, kt, :],
                                 K2T8[:, kt, :, :],
                                 FW8[:, :, qb:qb + 512],
                                 start=True, stop=True, perf_mode=DR)
            nc.scalar.activation(out=A28[:, :, qb:qb + 512], in_=s2[:, :, :],
                                 func=AF.Exp, scale=a2scale)

        def a2_out(qb):
            d2 = ps2_acc.tile([128, 512], F32, tag="acc2", name="d2")
            nc.tensor.matmul(d2[:, :], ONES2[:, :, :], A28[:, :, qb:qb + 512],
                             start=True, stop=True, perf_mode=DR)
            dr2 = opool.tile([128, 512], F32, tag="dr2", name="dr2")
            nc.vector.reciprocal(dr2[:, :], d2[:, :])
            for mt in range(2):
                f2 = ps2_acc.tile([128, 512], F32, tag="acc2", name="f2")
                nc.tensor.matmul(f2[:, :],
                                 K2K8[:, mt, :, :],
                                 A28[:, :, qb:qb + 512],
                                 start=True, stop=True, perf_mode=DR)
                tmp = opool.tile([128, 512], F32, tag="tmp", name="tmp")
                nc.vector.scalar_tensor_tensor(tmp[:, :], f2[:, :],
                                               1.0 / K2SC, dr2[:, :],
                                               op0=ALU.mult, op1=ALU.mult)
                outc = opool.tile([128, 512], F32, tag="outc", name="outc")
                nc.vector.tensor_add(outc[:, :], tmp[:, :],
                                     FW[:, mt, qb:qb + 512])
                nc.sync.dma_start(
                    out=o_d[mt * 128:(mt + 1) * 128, qb:qb + 512],
                    in_=outc[:, :])

        a2_scores(0)
        a2_scores(512)
        for qb in range(0, HW, 512):
            a2_out(qb)
            if qb + 1024 < HW:
                a2_scores(qb + 1024)


_NC_CACHE = None


def _get_nc():
    global _NC_CACHE
    if _NC_CACHE is None:
        _NC_CACHE = _build()
    return _NC_CACHE


def _prep_inputs(inputs):
    f = np.float32
    f8 = ml_dtypes.float8_e4m3fn
    s = 1.0 / math.sqrt(HD)
    x = np.asarray(inputs["x"], f).reshape(B, C, HW)

    # channel permutation for the Q/K DR-pair layout:
    # tile-group g, partition p = 32j + k (k<16 used, k>=16 duplicates), slot
    # s2 -> channel 32*(4g+j) + (p%32)%16 + 16*s2
    p_arr = np.arange(128)
    j_arr = p_arr // 32
    k_arr = (p_arr % 32) % 16
    cperm = np.zeros((2, 2, 128), np.int64)   # [g, s2, p] -> channel
    for g in range(2):
        for s2 in range(2):
            cperm[g, s2] = 32 * (4 * g + j_arr) + k_arr + 16 * s2

    def qk_w(wmat, scale):
        # wT[cin, cout] = wmat.T * scale; out [128, g, s2, ct, 128] fp8
        # (ct slot-pair contiguous for the dual-fp8 ldweights)
        wT = (np.asarray(wmat, f).T * scale)
        out = np.zeros((128, 2, 2, 2, 128), f)
        for g in range(2):
            for s2 in range(2):
                for ct in range(2):
                    out[:, g, s2, ct, :] = wT[128 * ct:128 * (ct + 1),
                                              cperm[g, s2]]
        return out.reshape(128, -1).astype(f8)

    def qk_b(bvec, scale):
        b = np.asarray(bvec, f) * scale
        out = np.zeros((128, 2, 2), f)
        for g in range(2):
            for s2 in range(2):
                out[:, g, s2] = b[cperm[g, s2]]
        return out.reshape(128, 4).copy()

    def dr_w(wmat):
        # moving-side layout [cin-pair partitions, (ct, cout)] fp8
        wT = np.asarray(wmat, f).T  # [cin, cout]
        return np.ascontiguousarray(
            wT.reshape(2, 128, C).transpose(1, 0, 2).reshape(128, 2 * C)
        ).astype(f8)

    def dr_wl(wmat):
        # stationary-side layout [cin-pair partitions, (cout-tile, ct, d)]
        wT = np.asarray(wmat, f).T  # [cin, cout]
        out = np.zeros((128, 2, 2, 128), f)
        for mt in range(2):
            for ct in range(2):
                out[:, mt, ct, :] = wT[128 * ct:128 * (ct + 1),
                                       128 * mt:128 * (mt + 1)]
        return out.reshape(128, -1).astype(f8)

    base = {
        "wq8": qk_w(inputs["Wq"], s),
        "wk8": qk_w(inputs["Wk"], 1.0),
        "wv8": dr_w(inputs["Wv"]),
        "wo8": dr_wl(inputs["Wo"]),
        "wf8l": dr_wl(inputs["Wfuse"]),
        "wf8r": dr_w(inputs["Wfuse"]),
        "bqp": qk_b(inputs["bq"], s),
        "bkp": qk_b(inputs["bk"], 1.0),
        "bvr": np.broadcast_to(np.asarray(inputs["bv"], f), (128, C)).copy(),
        "bo": np.asarray(inputs["bo"], f).reshape(C, 1).copy(),
        "bfp64": np.asarray(inputs["bfuse"], f).reshape(2, 128).T.copy() * K2SC,
        "bfr64": np.broadcast_to(np.asarray(inputs["bfuse"], f) * K2SC,
                                 (128, C)).copy(),
        "wrow_rep": np.broadcast_to(np.asarray(inputs["w_row"], f),
                                    (128, h2)).copy(),
        "brow_rep": np.broadcast_to(np.asarray(inputs["b_row"], f),
                                    (128, h2)).copy(),
        "wcol_rep": np.broadcast_to(np.asarray(inputs["w_col"], f),
                                    (128, h2)).copy(),
        "bcol_rep": np.broadcast_to(np.asarray(inputs["b_col"], f),
                                    (128, h2)).copy(),
        "wgt_rep": np.broadcast_to(np.asarray(inputs["w_gt"], f),
                                   (128, H)).copy(),
        "bgt_rep": np.broadcast_to(np.asarray(inputs["b_gt"], f),
                                   (128, H)).copy(),
    }
    return [dict(base, x=np.ascontiguousarray(x[b])) for b in range(B)]


def _run(inputs, **kwargs):
    nc = _get_nc()
    in_maps = _prep_inputs(inputs)
    return run_bass_kernel_spmd(nc, in_maps, core_ids=list(range(B)), **kwargs)


def kernel(**inputs) -> np.ndarray:
    res = _run(inputs)
    out = np.stack([r["o"] for r in res.results], axis=0)
    return out.reshape(B, C, H, W)


# revision 32
# speedup vs baseline: 1.0649x; 1.0649x over previous
"""Trainium2 Bass kernel for nn_BaseDTA (quadrant dual-token attention).

Data-parallel over batch: each of the 8 NeuronCores processes one sample
end-to-end (4 quadrant MHSA sequences of length 1026 + gating + a second
4096x256 cross-attention). No collectives.

v2: fp8e4 DoubleRow everywhere the math allows.
 - Q/K live in a "DR-pair" layout [128p, 2s, T]: head h = 4g+j owns
   partitions 32j..32j+16 of tile-group g; slot s holds channels 16s+k.
   Scores for one head are a single DR matmul (contraction 16x2=32).
 - V lives zero-padded per (g, j): V8Z[:, g, j, slot, 128ch] has head j's
   32 channels at cols 32j..32j+32, zeros elsewhere, so 4 heads accumulate
   into one full-128-row PSUM bank (DR col-packed outputs are rejected by
   walrus, zero-padding is free under the cost model).
 - Keys tile into 10 slots of 128 (slot 8 holds the 2 lt/gt keys, slot 9 is
   zero padding) -> 5 DoubleRow key-groups of 256 for AV/denominator.
 - Softmax denominators via ONESZ (ones at head j's columns) DR matmuls.
 - exp runs on ACT out of a 6-bank PSUM ring straight into fp8 At tiles.
 - attn2 K/V tensors are scaled x64 before fp8 (they sit near e4m3's
   subnormal range); the scale is folded into exp(scale=) and the output
   normalization.
"""

import math
from collections import deque

import numpy as np
import ml_dtypes

import concourse.bass as bass
import concourse.mybir as mybir
import concourse.tile as tile
from concourse import bacc
from concourse.bass_utils import run_bass_kernel_spmd

F32 = mybir.dt.float32
BF16 = mybir.dt.bfloat16
FP8 = mybir.dt.float8e4
AF = mybir.ActivationFunctionType
AX = mybir.AxisListType
ALU = mybir.AluOpType
DR = mybir.MatmulPerfMode.DoubleRow

B, C, H, W = 8, 256, 64, 64
h2, w2 = H // 2, W // 2          # 32
NH = 8
HD = C // NH                     # 32
HW = H * W                       # 4096
PIX = h2 * w2                    # 1024
T = PIX + 2                      # 1026
QC = 342                         # query chunk (3 per T)
NKT = 9                          # real key tiles (8x128 + 2)
NSLOT = 10                       # key slots incl zero pad
NKG = 5                          # DR key groups of 256
RING = 6                         # sp ring banks
K2SC = 64.0                      # attn2 K scaling before fp8


def _build():
    nc = bacc.Bacc(trn_type="TRN2", target_bir_lowering=False, num_devices=8)

    x_d = nc.dram_tensor("x", [C, HW], F32, kind="ExternalInput")
    wq8_d = nc.dram_tensor("wq8", [128, 2 * 2 * 2 * 128], FP8, kind="ExternalInput")
    wk8_d = nc.dram_tensor("wk8", [128, 2 * 2 * 2 * 128], FP8, kind="ExternalInput")
    wv8_d = nc.dram_tensor("wv8", [128, 2 * C], FP8, kind="ExternalInput")
    wo8_d = nc.dram_tensor("wo8", [128, 2 * C], FP8, kind="ExternalInput")
    wf8l_d = nc.dram_tensor("wf8l", [128, 2 * C], FP8, kind="ExternalInput")
    wf8r_d = nc.dram_tensor("wf8r", [128, 2 * C], FP8, kind="ExternalInput")
    bqp_d = nc.dram_tensor("bqp", [128, 4], F32, kind="ExternalInput")
    bkp_d = nc.dram_tensor("bkp", [128, 4], F32, kind="ExternalInput")
    bvr_d = nc.dram_tensor("bvr", [128, C], F32, kind="ExternalInput")
    bo_d = nc.dram_tensor("bo", [C, 1], F32, kind="ExternalInput")
    bfp64_d = nc.dram_tensor("bfp64", [128, 2], F32, kind="ExternalInput")
    bfr64_d = nc.dram_tensor("bfr64", [128, C], F32, kind="ExternalInput")
    g_names = ["wrow_rep", "brow_rep", "wcol_rep", "bcol_rep"]
    g_d = {n: nc.dram_tensor(n, [128, h2], F32, kind="ExternalInput") for n in g_names}
    wgt_rep_d = nc.dram_tensor("wgt_rep", [128, H], F32, kind="ExternalInput")
    bgt_rep_d = nc.dram_tensor("bgt_rep", [128, H], F32, kind="ExternalInput")
    o_d = nc.dram_tensor("o", [C, HW], F32, kind="ExternalOutput")

    with tile.TileContext(nc) as tc:
        _emit(nc, tc, x_d, wq8_d, wk8_d, wv8_d, wo8_d, wf8l_d, wf8r_d, bqp_d, bkp_d,
              bvr_d, bo_d, bfp64_d, bfr64_d, g_d, wgt_rep_d, bgt_rep_d, o_d)
    nc.compile()
    return nc


def _emit(nc, tc, x_d, wq8_d, wk8_d, wv8_d, wo8_d, wf8l_d, wf8r_d, bqp_d, bkp_d,
          bvr_d, bo_d, bfp64_d, bfr64_d, g_d, wgt_rep_d, bgt_rep_d, o_d):
    with tc.tile_pool(name="singles", bufs=1) as sg:
        X = sg.tile([128, 2, HW], F32)
        FW = sg.tile([128, 2, HW], F32)
        RP = sg.tile([128, 2, 4 * H], F32)
        WQ8f = sg.tile([128, 2 * 2 * 2 * 128], FP8)
        WK8f = sg.tile([128, 2 * 2 * 2 * 128], FP8)
        WV8f = sg.tile([128, 2 * C], FP8)
        WO8f = sg.tile([128, 2 * C], FP8)
        WF8Lf = sg.tile([128, 2 * C], FP8)
        WF8Rf = sg.tile([128, 2 * C], FP8)
        # all lhsT layouts keep the ct slot-pair contiguous (dual-fp8
        # ldweights requires slot stride == inner size)
        WQ8 = WQ8f.rearrange("p (g s ct d) -> p g s ct d", g=2, s=2, ct=2)
        WK8 = WK8f.rearrange("p (g s ct d) -> p g s ct d", g=2, s=2, ct=2)
        WV8 = WV8f.rearrange("p (ct d) -> p ct d", ct=2)
        WO8 = WO8f.rearrange("p (mt ct d) -> p mt ct d", mt=2, ct=2)
        WF8L = WF8Lf.rearrange("p (s ct d) -> p s ct d", s=2, ct=2)
        WF8R = WF8Rf.rearrange("p (ct d) -> p ct d", ct=2)
        BQP = sg.tile([128, 4], F32)
        BKP = sg.tile([128, 4], F32)
        BVr = sg.tile([128, C], F32)
        BO = sg.tile([128, 2, 1], F32)
        BFP64 = sg.tile([128, 2], F32)
        BFr64 = sg.tile([128, C], F32)
        WRr = sg.tile([128, h2], F32)
        BRr = sg.tile([128, h2], F32)
        WCr = sg.tile([128, h2], F32)
        BCr = sg.tile([128, h2], F32)
        WGr = sg.tile([128, H], F32)
        BGr = sg.tile([128, H], F32)
        GT = sg.tile([128, 2, 1], F32)
        ONESZ = sg.tile([128, 4, 2, 128], FP8)    # dp lhsT, m<4
        ONESZ4 = sg.tile([128, 4, 2, 128], FP8)   # dp lhsT, m=4 (2 real keys)
        ONES2 = sg.tile([128, 2, 128], FP8)       # attn2 d2 lhsT

        for ct in range(2):
            for xc in range(4):
                nc.sync.dma_start(
                    out=X[:, ct, xc * 1024:(xc + 1) * 1024],
                    in_=x_d[ct * 128:(ct + 1) * 128, xc * 1024:(xc + 1) * 1024])
        nc.sync.dma_start(out=WQ8f[:, :], in_=wq8_d[:, :])
        nc.sync.dma_start(out=WK8f[:, :], in_=wk8_d[:, :])
        nc.sync.dma_start(out=WV8f[:, :], in_=wv8_d[:, :])
        nc.sync.dma_start(out=WO8f[:, :], in_=wo8_d[:, :])
        nc.sync.dma_start(out=WF8Lf[:, :], in_=wf8l_d[:, :])
        nc.sync.dma_start(out=WF8Rf[:, :], in_=wf8r_d[:, :])
        nc.sync.dma_start(out=BQP[:, :], in_=bqp_d[:, :])
        nc.sync.dma_start(out=BKP[:, :], in_=bkp_d[:, :])
        nc.sync.dma_start(out=BVr[:, :], in_=bvr_d[:, :])
        for mt in range(2):
            nc.sync.dma_start(out=BO[:, mt, :],
                              in_=bo_d[mt * 128:(mt + 1) * 128, :])
        nc.sync.dma_start(out=BFP64[:, :], in_=bfp64_d[:, :])
        nc.sync.dma_start(out=BFr64[:, :], in_=bfr64_d[:, :])
        for n, dst in [("wrow_rep", WRr), ("brow_rep", BRr),
                       ("wcol_rep", WCr), ("bcol_rep", BCr)]:
            nc.sync.dma_start(out=dst[:, :], in_=g_d[n][:, :])
        nc.sync.dma_start(out=WGr[:, :], in_=wgt_rep_d[:, :])
        nc.sync.dma_start(out=BGr[:, :], in_=bgt_rep_d[:, :])

        # one-time constant tiles: built on the (otherwise idle) Pool engine
        nc.gpsimd.memset(ONESZ[:, :, :, :], 0.0)
        nc.gpsimd.memset(ONESZ4[:, :, :, :], 0.0)
        nc.gpsimd.memset(ONES2[:, :, :], 1.0)
        for j in range(4):
            nc.gpsimd.memset(ONESZ[:, j, :, 32 * j:32 * j + 32], 1.0)
            nc.gpsimd.memset(ONESZ4[0:2, j, 0, 32 * j:32 * j + 32], 1.0)

        with (
            tc.tile_pool(name="ypool", bufs=2) as ypool,
            tc.tile_pool(name="y8pool", bufs=1) as y8pool,
            tc.tile_pool(name="qkpool", bufs=2) as qkpool,
            tc.tile_pool(name="v8zpool", bufs=2) as v8zpool,
            tc.tile_pool(name="atpool", bufs=2) as atpool,
            tc.tile_pool(name="aopool", bufs=1) as aopool,
            tc.tile_pool(name="scratch", bufs=2) as scratch,
            tc.tile_pool(name="gpool", bufs=1) as gpool,
            tc.tile_pool(name="ps_ring", bufs=1, space="PSUM") as ps_ring,
            tc.tile_pool(name="ps_acc", bufs=2, space="PSUM") as ps_acc,
        ):
            # pre-zero pad regions once per physical buffer (on Pool)
            for _ in range(2):
                At = atpool.tile([128, NSLOT, NH, QC], FP8, tag="At", name="At")
                nc.gpsimd.memset(At[:, 8:10, :, :], 0.0)
            for _ in range(2):
                V8Z = v8zpool.tile([128, 2, 4, NSLOT, 128], FP8, tag="V8Z",
                                   name="V8Z")
                nc.gpsimd.memset(V8Z[:, :, :, :, :], 0.0)
            V8 = v8zpool.tile([128, NSLOT, C], FP8, tag="V8", bufs=1,
                              name="V8")
            nc.gpsimd.memset(V8[:, :, :], 0.0)

            # global token: mean over all pixels, on ACT (accum_out) so the
            # startup DVE chain stays short
            GTP = scratch.tile([128, 2, 4], F32, tag="GTP", bufs=1, name="GTP")
            for ct in range(2):
                for xc in range(4):
                    scr = gpool.tile([128, 1024], BF16, tag="prod", name="scr")
                    nc.scalar.activation(out=scr[:, :],
                                         in_=X[:, ct, xc * 1024:(xc + 1) * 1024],
                                         func=AF.Copy,
                                         accum_out=GTP[:, ct, xc:xc + 1])
                nc.vector.reduce_sum(GT[:, ct, :], GTP[:, ct, :], AX.X)
                nc.vector.tensor_scalar_mul(GT[:, ct, :], GT[:, ct, :],
                                            1.0 / HW)

            quad_state = {}
            # Shared FIFO of deferred emissions. Every user of the shared
            # "acc" PSUM pool allocates its tile INSIDE its closure, so
            # allocation order == emission order and the 2-slot rotation
            # stays correctly nested.
            backlog = deque()

            def flush(k):
                for _ in range(min(k, len(backlog))):
                    backlog.popleft()()

            def acc_tile(name):
                return ps_acc.tile([128, 512], F32, tag="acc", name=name)

            def prep(q):
                """Emit Y/Y8 and queue Q/K/V projections for quad q."""
                Y = ypool.tile([128, 2, T], F32, tag="Y", name="Y")
                Y8 = y8pool.tile([128, 2, T], FP8, tag="Y8", name="Y8")
                # token-blocked fp8 Y for the V-proj lhsT (slot pair = ct must
                # be contiguous); slot 8 holds lt/gt + zero pad
                Y8V = y8pool.tile([128, NKT, 2, 128], FP8, tag="Y8V",
                                  name="Y8V")
                QDR = qkpool.tile([128, 2, 2, T], FP8, tag="QDR", name="QDR")
                # key-blocked K: [p, g, kt, ct-slot, 128]; kt=8 cols 2..128
                # are zero-padded fake keys (their exp=1 rows are masked by
                # V8Z/ONESZ4 zeros)
                KDR = qkpool.tile([128, 2, NKT, 2, 128], FP8, tag="KDR",
                                  name="KDR")
                V8Z = v8zpool.tile([128, 2, 4, NSLOT, 128], FP8, tag="V8Z",
                                   name="V8Zq")
                LTG = scratch.tile([128, 2, 2], F32, tag="LTG", name="LTG")
                quad_state[q] = dict(Y=Y, QDR=QDR, KDR=KDR, V8Z=V8Z, LTG=LTG)
                nc.vector.memset(Y8V[:, 8, :, 2:128], 0.0)
                nc.vector.memset(KDR[:, :, 8, :, 2:128], 0.0)
                r0, c0 = h2 * (q // 2), w2 * (q % 2)
                for ct in range(2):
                    xv = X[:, ct, :].rearrange("p (a b) -> p a b", a=H)[
                        :, r0:r0 + h2, c0:c0 + w2]
                    yq = Y[:, ct, 0:PIX].rearrange("p (a b) -> p a b", a=h2)
                    nc.vector.tensor_copy(yq, xv)
                    nc.vector.reduce_sum(LTG[:, ct, 0:1], Y[:, ct, 0:PIX], AX.X)
                    nc.vector.tensor_scalar_mul(LTG[:, ct, 0:1],
                                                LTG[:, ct, 0:1], 1.0 / PIX)
                    nc.vector.tensor_copy(LTG[:, ct, 1:2], GT[:, ct, :])
                    nc.vector.tensor_copy(Y[:, ct, PIX:T], LTG[:, ct, :])
                    nc.vector.tensor_copy(Y8[:, ct, :], Y[:, ct, :])
                    nc.vector.tensor_copy(
                        Y8V[:, 0:8, ct, :],
                        Y[:, ct, 0:PIX].rearrange("p (a b) -> p a b", b=128))
                    nc.vector.tensor_copy(Y8V[:, 8, ct, 0:2],
                                          Y[:, ct, PIX:T])
                    yield
                # Q projection (flat layout, rhs-only -> strided slots fine)
                for g in range(2):
                    for s in range(2):
                        for qs in range(0, T, QC):
                            def qp(g=g, s=s, qs=qs, Y8=Y8, QDR=QDR):
                                pq = acc_tile("pq")
                                nc.tensor.matmul(
                                    pq[:, 0:QC], WQ8[:, g, s, :, :],
                                    Y8[:, :, qs:qs + QC],
                                    start=True, stop=True, perf_mode=DR)
                                nc.vector.tensor_scalar_add(
                                    QDR[:, g, s, qs:qs + QC], pq[:, 0:QC],
                                    BQP[:, 2 * g + s:2 * g + s + 1])
                            backlog.append(qp)
                            yield
                # K projection into the key-blocked layout (256-token chunks)
                for g in range(2):
                    for s in range(2):
                        for c4 in range(4):
                            def kp(g=g, s=s, c4=c4, Y8=Y8, KDR=KDR):
                                pq = acc_tile("pk")
                                nc.tensor.matmul(
                                    pq[:, 0:256], WK8[:, g, s, :, :],
                                    Y8[:, :, 256 * c4:256 * c4 + 256],
                                    start=True, stop=True, perf_mode=DR)
                                nc.vector.tensor_scalar_add(
                                    KDR[:, g, 2 * c4:2 * c4 + 2, s, :],
                                    pq[:, 0:256].rearrange(
                                        "p (a b) -> p a b", b=128),
                                    BKP[:, 2 * g + s:2 * g + s + 1])
                            backlog.append(kp)
                            yield
                        def kp_tail(g=g, s=s, Y8=Y8, KDR=KDR):
                            pq = acc_tile("pkt")
                            nc.tensor.matmul(
                                pq[:, 0:2], WK8[:, g, s, :, :],
                                Y8[:, :, 1024:1026],
                                start=True, stop=True, perf_mode=DR)
                            nc.vector.tensor_scalar_add(
                                KDR[:, g, 8, s, 0:2], pq[:, 0:2],
                                BKP[:, 2 * g + s:2 * g + s + 1])
                        backlog.append(kp_tail)
                        yield
                for tt in range(NKT):
                    def vp(tt=tt, Y8V=Y8V):
                        n = 128 if tt < 8 else T - 8 * 128
                        pv = acc_tile("pv")
                        nc.tensor.matmul(pv[:, 0:C], Y8V[:, tt, :, :],
                                         WV8[:, :, :],
                                         start=True, stop=True, perf_mode=DR)
                        nc.vector.tensor_add(V8[0:n, tt, :], pv[0:n, 0:C],
                                             BVr[0:n, :])
                    backlog.append(vp)
                    yield

                def v8z_fill(V8Z=V8Z):
                    # scatter V8 into the zero-padded per-(g, j) lhsT layout
                    # on the Pool engine (keeps DVE free)
                    for g in range(2):
                        for j in range(4):
                            cs = 128 * g + 32 * j
                            nc.gpsimd.tensor_copy(
                                V8Z[:, g, j, :, 32 * j:32 * j + 32],
                                V8[:, :, cs:cs + 32])
                backlog.append(v8z_fill)
                yield

            def att(q):
                """Attention for quad q. Yields after each kt exp so the
                driver can interleave prep(q+1) work."""
                r0, c0 = h2 * (q // 2), w2 * (q % 2)
                st = quad_state[q]
                Y, QDR, KDR, V8Z, LTG = (st["Y"], st["QDR"], st["KDR"],
                                         st["V8Z"], st["LTG"])
                AT = aopool.tile([128, 2, T], FP8, tag="AT", name="AT")
                OT = aopool.tile([128, 2, T], F32, tag="OT", name="OT")

                sp = ps_ring.tile([128, RING, 512], F32, tag="sp", name="sp")
                wcount = 0
                for qs in range(0, T, QC):
                    At = atpool.tile([128, NSLOT, NH, QC], FP8, tag="At",
                                     name="Atq")
                    for kt in range(NKT):
                        # 3 windows of heads (3, 3, 2); windows alternate
                        # between bank triples [0-2] and [3-5] (reuse is
                        # always two windows behind = double buffered)
                        for h0, h1 in ((0, 3), (3, 6), (6, 8)):
                            pos = 3 * (wcount % 2)
                            wcount += 1
                            for i, h in enumerate(range(h0, h1)):
                                g, j = h // 4, h % 4
                                nc.tensor.matmul(
                                    sp[:, pos + i, 0:QC],
                                    KDR[32 * j:32 * j + 16, g, kt, :, :],
                                    QDR[32 * j:32 * j + 16, g, :, qs:qs + QC],
                                    start=True, stop=True, perf_mode=DR,
                                    tile_position=(32 * j, 0))
                            # deferred PE work goes AFTER this window's
                            # scores (so exp never waits on it) and BEFORE
                            # the next window's scores (small items only)
                            flush(3)
                            nc.scalar.activation(
                                out=At[:, kt, h0:h1, :],
                                in_=sp[:, pos:pos + (h1 - h0), 0:QC],
                                func=AF.Exp)
                            yield

                    for g in range(2):
                        # queue AV + dp + normalize for this (g, qs); the
                        # avp/dp banks are allocated at flush time (m == 0)
                        # to keep the acc-pool rotation FIFO
                        hold = {}

                        def mk_av(m, j, g=g, At=At, hold=hold, V8Z=V8Z):
                            def go():
                                if m == 0 and j == 0:
                                    hold["avp"] = acc_tile("avp")
                                    hold["dp"] = acc_tile("dp")
                                avp, dp = hold["avp"], hold["dp"]
                                nc.tensor.matmul(
                                    avp[:, 0:QC],
                                    V8Z[:, g, j, 2 * m:2 * m + 2, :],
                                    At[:, 2 * m:2 * m + 2, 4 * g + j, :],
                                    start=(m == 0 and j == 0),
                                    stop=(m == 4 and j == 3),
                                    perf_mode=DR)
                                oz = ONESZ if m < 4 else ONESZ4
                                nc.tensor.matmul(
                                    dp[:, 0:QC],
                                    oz[:, j, :, :],
                                    At[:, 2 * m:2 * m + 2, 4 * g + j, :],
                                    start=(m == 0 and j == 0),
                                    stop=(m == 4 and j == 3),
                                    perf_mode=DR)
                            return go

                        for m in range(NKG):
                            for j in range(4):
                                backlog.append(mk_av(m, j))

                        def norm(g=g, qs=qs, hold=hold, AT=AT):
                            dr = scratch.tile([128, QC], F32, tag="dr",
                                              name="dr")
                            nc.vector.reciprocal(dr[:, :], hold["dp"][:, 0:QC])
                            nc.vector.tensor_mul(AT[:, g, qs:qs + QC],
                                                 hold["avp"][:, 0:QC],
                                                 dr[:, :])
                        backlog.append(norm)

                        if g == 1:
                            def oproj(qs=qs, AT=AT, OT=OT, Y=Y):
                                for mt in range(2):
                                    po = acc_tile("po")
                                    nc.tensor.matmul(
                                        po[:, 0:QC],
                                        WO8[:, mt, :, :],
                                        AT[:, :, qs:qs + QC],
                                        start=True, stop=True, perf_mode=DR)
                                    nc.vector.scalar_tensor_tensor(
                                        OT[:, mt, qs:qs + QC], po[:, 0:QC],
                                        BO[:, mt, :], Y[:, mt, qs:qs + QC],
                                        op0=ALU.add, op1=ALU.add)
                            backlog.append(oproj)
                while backlog:
                    backlog.popleft()()
                    yield

                # gating
                for ct in range(2):
                    ltp = OT[:, ct, PIX:PIX + 1]
                    gtp = OT[:, ct, PIX + 1:T]
                    row = gpool.tile([128, h2], F32, tag="row", name="row")
                    col = gpool.tile([128, h2], F32, tag="col", name="col")
                    nc.vector.scalar_tensor_tensor(row[:, :], WRr[:, :], ltp,
                                                   BRr[:, :], op0=ALU.mult,
                                                   op1=ALU.add)
                    nc.vector.scalar_tensor_tensor(col[:, :], WCr[:, :], ltp,
                                                   BCr[:, :], op0=ALU.mult,
                                                   op1=ALU.add)
                    prod = gpool.tile([128, h2, w2], BF16, tag="prod",
                                      name="prod")
                    nc.vector.tensor_mul(
                        prod[:, :, :],
                        row[:, :, None].broadcast_to([128, h2, w2]),
                        col[:, None, :].broadcast_to([128, h2, w2]))
                    eg = gpool.tile([128, h2, w2], BF16, tag="eg", name="eg")
                    nc.scalar.activation(out=eg[:, :, :], in_=prod[:, :, :],
                                         func=AF.Exp)
                    # sigmoid(z) = e/(1+e): stays on the Exp ACT table
                    e1 = gpool.tile([128, h2, w2], BF16, tag="prod", name="e1")
                    nc.vector.tensor_scalar_add(e1[:, :, :], eg[:, :, :], 1.0)
                    with nc.allow_low_precision(reason="sigmoid gate in bf16"):
                        nc.vector.reciprocal(e1[:, :, :], e1[:, :, :])
                    nc.vector.tensor_mul(eg[:, :, :], eg[:, :, :], e1[:, :, :])
                    fv = FW[:, ct, :].rearrange("p (a b) -> p a b", a=H)[
                        :, r0:r0 + h2, c0:c0 + w2]
                    xp = OT[:, ct, 0:PIX].rearrange("p (a b) -> p a b", a=h2)
                    nc.vector.tensor_mul(fv, xp, eg[:, :, :])
                    rp_v = RP[:, ct, :].rearrange("p (a b) -> p a b", b=4)[
                        :, :, q]
                    nc.vector.scalar_tensor_tensor(rp_v, WGr[:, :], gtp,
                                                   BGr[:, :], op0=ALU.mult,
                                                   op1=ALU.add)
                    yield

            def drain(gen):
                if gen is None:
                    return
                for _ in gen:
                    pass

            drain(prep(0))
            # prep(0)'s projection closures must be emitted before att(0)
            # emits scores that read QDR/KDR (no writer yet = no dependency)
            while backlog:
                backlog.popleft()()
            for q in range(4):
                a = att(q)
                p = prep(q + 1) if q < 3 else None
                while True:
                    try:
                        next(a)
                    except StopIteration:
                        break
                    if p is not None:
                        for _ in range(2):
                            try:
                                next(p)
                            except StopIteration:
                                p = None
                                break
                drain(p)

        _emit_attn2(nc, tc, FW, RP, WF8L, WF8R, BFP64, BFr64, ONES2, o_d)


def _emit_attn2(nc, tc, FW, RP, WF8L, WF8R, BFP64, BFr64, ONES2, o_d):
    with (
        tc.tile_pool(name="a2pool", bufs=1) as a2pool,
        tc.tile_pool(name="opool", bufs=2) as opool,
        tc.tile_pool(name="ps2_s", bufs=2, space="PSUM") as ps2_s,
        tc.tile_pool(name="ps2_acc", bufs=2, space="PSUM") as ps2_acc,
    ):
        FW8 = a2pool.tile([128, 2, HW], FP8, tag="FW8", name="FW8")
        RP8R = a2pool.tile([128, 2, 4 * H], FP8, tag="RP8R", name="RP8R")
        RP8L = a2pool.tile([128, 2, 2, 128], FP8, tag="RP8L", name="RP8L")
        K2T8 = a2pool.tile([128, 2, 2, 128], FP8, tag="K2T8", name="K2T8")
        K2K8 = a2pool.tile([128, 2, 2, 128], FP8, tag="K2K8", name="K2K8")
        A28 = a2pool.tile([128, 2, HW], FP8, tag="A28", name="A28")

        for ct in range(2):
            nc.vector.tensor_copy(FW8[:, ct, :], FW[:, ct, :])
            nc.vector.tensor_scalar_mul(RP8R[:, ct, :], RP[:, ct, :], K2SC)
            nc.vector.tensor_scalar_mul(
                RP8L[:, :, ct, :],
                RP[:, ct, :].rearrange("p (a b) -> p a b", b=128), K2SC)

        # K2T8[p, kt, s, key] = 64*K2T[cout=128s+p, key=128kt+key]
        # K2K8[p, mt, s, c] = 64*K2K[key=128s+p, cout=128mt+c]
        for s in range(2):
            pk = ps2_acc.tile([128, 512], F32, tag="acc2", name="pk")
            nc.tensor.matmul(pk[:, 0:C], WF8L[:, s, :, :],
                             RP8R[:, :, :], start=True, stop=True,
                             perf_mode=DR)
            nc.vector.tensor_scalar_add(
                K2T8[:, :, s, :],
                pk[:, 0:C].rearrange("p (a b) -> p a b", b=128),
                BFP64[:, s:s + 1])
        for s in range(2):
            pk = ps2_acc.tile([128, 512], F32, tag="acc2", name="pk2")
            nc.tensor.matmul(pk[:, 0:C], RP8L[:, s, :, :],
                             WF8R[:, :, :], start=True, stop=True,
                             perf_mode=DR)
            nc.vector.tensor_add(
                K2K8[:, :, s, :],
                pk[:, 0:C].rearrange("p (a b) -> p a b", b=128),
                BFr64[:, :].rearrange("p (a b) -> p a b", b=128))

        a2scale = 1.0 / (math.sqrt(C) * K2SC)

        def a2_scores(qb):
            s2 = ps2_s.tile([128, 2, 512], F32, tag="s2", name="s2")
            for kt in range(2):
                nc.tensor.matmul(s2[:, kt, :],
                                 K2T8[:, kt, :, :],
                                 FW8[:, :, qb:qb + 512],
                                 start=True, stop=True, perf_mode=DR)
            nc.scalar.activation(out=A28[:, :, qb:qb + 512], in_=s2[:, :, :],
                                 func=AF.Exp, scale=a2scale)

        def a2_out(qb):
            d2 = ps2_acc.tile([128, 512], F32, tag="acc2", name="d2")
            nc.tensor.matmul(d2[:, :], ONES2[:, :, :], A28[:, :, qb:qb + 512],
                             start=True, stop=True, perf_mode=DR)
            dr2 = opool.tile([128, 512], F32, tag="dr2", name="dr2")
            nc.vector.reciprocal(dr2[:, :], d2[:, :])
            for mt in range(2):
                f2 = ps2_acc.tile([128, 512], F32, tag="acc2", name="f2")
                nc.tensor.matmul(f2[:, :],
                                 K2K8[:, mt, :, :],
                                 A28[:, :, qb:qb + 512],
                                 start=True, stop=True, perf_mode=DR)
                tmp = opool.tile([128, 512], F32, tag="tmp", name="tmp")
                nc.vector.scalar_tensor_tensor(tmp[:, :], f2[:, :],
                                               1.0 / K2SC, dr2[:, :],
                                               op0=ALU.mult, op1=ALU.mult)
                outc = opool.tile([128, 512], F32, tag="outc", name="outc")
                nc.vector.tensor_add(outc[:, :], tmp[:, :],
                                     FW[:, mt, qb:qb + 512])
                nc.sync.dma_start(
                    out=o_d[mt * 128:(mt + 1) * 128, qb:qb + 512],
                    in_=outc[:, :])

        a2_scores(0)
        a2_scores(512)
        for qb in range(0, HW, 512):
            a2_out(qb)
            if qb + 1024 < HW:
                a2_scores(qb + 1024)


_NC_CACHE = None


def _get_nc():
    global _NC_CACHE
    if _NC_CACHE is None:
        _NC_CACHE = _build()
    return _NC_CACHE


def _prep_inputs(inputs):
    f = np.float32
    f8 = ml_dtypes.float8_e4m3fn
    s = 1.0 / math.sqrt(HD)
    x = np.asarray(inputs["x"], f).reshape(B, C, HW)

    # channel permutation for the Q/K DR-pair layout:
    # tile-group g, partition p = 32j + k (k<16 used, k>=16 duplicates), slot
    # s2 -> channel 32*(4g+j) + (p%32)%16 + 16*s2
    p_arr = np.arange(128)
    j_arr = p_arr // 32
    k_arr = (p_arr % 32) % 16
    cperm = np.zeros((2, 2, 128), np.int64)   # [g, s2, p] -> channel
    for g in range(2):
        for s2 in range(2):
            cperm[g, s2] = 32 * (4 * g + j_arr) + k_arr + 16 * s2

    def qk_w(wmat, scale):
        # wT[cin, cout] = wmat.T * scale; out [128, g, s2, ct, 128] fp8
        # (ct slot-pair contiguous for the dual-fp8 ldweights)
        wT = (np.asarray(wmat, f).T * scale)
        out = np.zeros((128, 2, 2, 2, 128), f)
        for g in range(2):
            for s2 in range(2):
                for ct in range(2):
                    out[:, g, s2, ct, :] = wT[128 * ct:128 * (ct + 1),
                                              cperm[g, s2]]
        return out.reshape(128, -1).astype(f8)

    def qk_b(bvec, scale):
        b = np.asarray(bvec, f) * scale
        out = np.zeros((128, 2, 2), f)
        for g in range(2):
            for s2 in range(2):
                out[:, g, s2] = b[cperm[g, s2]]
        return out.reshape(128, 4).copy()

    def dr_w(wmat):
        # moving-side layout [cin-pair partitions, (ct, cout)] fp8
        wT = np.asarray(wmat, f).T  # [cin, cout]
        return np.ascontiguousarray(
            wT.reshape(2, 128, C).transpose(1, 0, 2).reshape(128, 2 * C)
        ).astype(f8)

    def dr_wl(wmat):
        # stationary-side layout [cin-pair partitions, (cout-tile, ct, d)]
        wT = np.asarray(wmat, f).T  # [cin, cout]
        out = np.zeros((128, 2, 2, 128), f)
        for mt in range(2):
            for ct in range(2):
                out[:, mt, ct, :] = wT[128 * ct:128 * (ct + 1),
                                       128 * mt:128 * (mt + 1)]
        return out.reshape(128, -1).astype(f8)

    base = {
        "wq8": qk_w(inputs["Wq"], s),
        "wk8": qk_w(inputs["Wk"], 1.0),
        "wv8": dr_w(inputs["Wv"]),
        "wo8": dr_wl(inputs["Wo"]),
        "wf8l": dr_wl(inputs["Wfuse"]),
        "wf8r": dr_w(inputs["Wfuse"]),
        "bqp": qk_b(inputs["bq"], s),
        "bkp": qk_b(inputs["bk"], 1.0),
        "bvr": np.broadcast_to(np.asarray(inputs["bv"], f), (128, C)).copy(),
        "bo": np.asarray(inputs["bo"], f).reshape(C, 1).copy(),
        "bfp64": np.asarray(inputs["bfuse"], f).reshape(2, 128).T.copy() * K2SC,
        "bfr64": np.broadcast_to(np.asarray(inputs["bfuse"], f) * K2SC,
                                 (128, C)).copy(),
        "wrow_rep": np.broadcast_to(np.asarray(inputs["w_row"], f),
                                    (128, h2)).copy(),
        "brow_rep": np.broadcast_to(np.asarray(inputs["b_row"], f),
                                    (128, h2)).copy(),
        "wcol_rep": np.broadcast_to(np.asarray(inputs["w_col"], f),
                                    (128, h2)).copy(),
        "bcol_rep": np.broadcast_to(np.asarray(inputs["b_col"], f),
                                    (128, h2)).copy(),
        "wgt_rep": np.broadcast_to(np.asarray(inputs["w_gt"], f),
                                   (128, H)).copy(),
        "bgt_rep": np.broadcast_to(np.asarray(inputs["b_gt"], f),
                                   (128, H)).copy(),
    }
    return [dict(base, x=np.ascontiguousarray(x[b])) for b in range(B)]


def _run(inputs, **kwargs):
    nc = _get_nc()
    in_maps = _prep_inputs(inputs)
    return run_bass_kernel_spmd(nc, in_maps, core_ids=list(range(B)), **kwargs)


def kernel(**inputs) -> np.ndarray:
    res = _run(inputs)
    out = np.stack([r["o"] for r in res.results], axis=0)
    return out.reshape(B, C, H, W)


# revision 33
# speedup vs baseline: 1.5603x; 1.4651x over previous
"""Trainium2 Bass kernel for nn_BaseDTA (quadrant dual-token attention).

Data-parallel over batch: each of the 8 NeuronCores processes one sample
end-to-end (4 quadrant MHSA sequences of length 1026 + gating + a second
4096x256 cross-attention). No collectives.

v2: fp8e4 DoubleRow everywhere the math allows.
 - Q/K live in a "DR-pair" layout [128p, 2s, T]: head h = 4g+j owns
   partitions 32j..32j+16 of tile-group g; slot s holds channels 16s+k.
   Scores for one head are a single DR matmul (contraction 16x2=32).
 - V lives zero-padded per (g, j): V8Z[:, g, j, slot, 128ch] has head j's
   32 channels at cols 32j..32j+32, zeros elsewhere, so 4 heads accumulate
   into one full-128-row PSUM bank (DR col-packed outputs are rejected by
   walrus, zero-padding is free under the cost model).
 - Keys tile into 10 slots of 128 (slot 8 holds the 2 lt/gt keys, slot 9 is
   zero padding) -> 5 DoubleRow key-groups of 256 for AV/denominator.
 - Softmax denominators via ONESZ (ones at head j's columns) DR matmuls.
 - exp runs on ACT out of a 6-bank PSUM ring straight into fp8 At tiles.
 - attn2 K/V tensors are scaled x64 before fp8 (they sit near e4m3's
   subnormal range); the scale is folded into exp(scale=) and the output
   normalization.
"""

import math
from collections import deque

import numpy as np
import ml_dtypes

import concourse.bass as bass
import concourse.mybir as mybir
import concourse.tile as tile
from concourse import bacc
from concourse.bass_utils import run_bass_kernel_spmd

F32 = mybir.dt.float32
BF16 = mybir.dt.bfloat16
FP8 = mybir.dt.float8e4
AF = mybir.ActivationFunctionType
AX = mybir.AxisListType
ALU = mybir.AluOpType
DR = mybir.MatmulPerfMode.DoubleRow

B, C, H, W = 8, 256, 64, 64
h2, w2 = H // 2, W // 2          # 32
NH = 8
HD = C // NH                     # 32
HW = H * W                       # 4096
PIX = h2 * w2                    # 1024
T = PIX + 2                      # 1026
QC = 342                         # query chunk (3 per T)
NKT = 9                          # real key tiles (8x128 + 2)
NSLOT = 10                       # key slots incl zero pad
NKG = 5                          # DR key groups of 256
RING = 6                         # sp ring banks
K2SC = 64.0                      # attn2 K scaling before fp8


def _build():
    nc = bacc.Bacc(trn_type="TRN2", target_bir_lowering=False, num_devices=8)

    x_d = nc.dram_tensor("x", [C, HW], F32, kind="ExternalInput")
    wq8_d = nc.dram_tensor("wq8", [128, 2 * 2 * 2 * 128], FP8, kind="ExternalInput")
    wk8_d = nc.dram_tensor("wk8", [128, 2 * 2 * 2 * 128], FP8, kind="ExternalInput")
    wv8_d = nc.dram_tensor("wv8", [128, 2 * C], FP8, kind="ExternalInput")
    wo8_d = nc.dram_tensor("wo8", [128, 2 * C], FP8, kind="ExternalInput")
    wf8l_d = nc.dram_tensor("wf8l", [128, 2 * C], FP8, kind="ExternalInput")
    wf8r_d = nc.dram_tensor("wf8r", [128, 2 * C], FP8, kind="ExternalInput")
    bqp_d = nc.dram_tensor("bqp", [128, 4], F32, kind="ExternalInput")
    bkp_d = nc.dram_tensor("bkp", [128, 4], F32, kind="ExternalInput")
    bvr_d = nc.dram_tensor("bvr", [128, C], F32, kind="ExternalInput")
    bo_d = nc.dram_tensor("bo", [C, 1], F32, kind="ExternalInput")
    bfp64_d = nc.dram_tensor("bfp64", [128, 2], F32, kind="ExternalInput")
    bfr64_d = nc.dram_tensor("bfr64", [128, C], F32, kind="ExternalInput")
    g_names = ["wrow_rep", "brow_rep", "wcol_rep", "bcol_rep"]
    g_d = {n: nc.dram_tensor(n, [128, h2], F32, kind="ExternalInput") for n in g_names}
    wgt_rep_d = nc.dram_tensor("wgt_rep", [128, H], F32, kind="ExternalInput")
    bgt_rep_d = nc.dram_tensor("bgt_rep", [128, H], F32, kind="ExternalInput")
    o_d = nc.dram_tensor("o", [C, HW], F32, kind="ExternalOutput")

    with tile.TileContext(nc) as tc:
        _emit(nc, tc, x_d, wq8_d, wk8_d, wv8_d, wo8_d, wf8l_d, wf8r_d, bqp_d, bkp_d,
              bvr_d, bo_d, bfp64_d, bfr64_d, g_d, wgt_rep_d, bgt_rep_d, o_d)
    nc.compile()
    return nc


def _emit(nc, tc, x_d, wq8_d, wk8_d, wv8_d, wo8_d, wf8l_d, wf8r_d, bqp_d, bkp_d,
          bvr_d, bo_d, bfp64_d, bfr64_d, g_d, wgt_rep_d, bgt_rep_d, o_d):
    with tc.tile_pool(name="singles", bufs=1) as sg:
        X = sg.tile([128, 2, HW], F32)
        FW = sg.tile([128, 2, HW], F32)
        RP = sg.tile([128, 2, 4 * H], F32)
        WQ8f = sg.tile([128, 2 * 2 * 2 * 128], FP8)
        WK8f = sg.tile([128, 2 * 2 * 2 * 128], FP8)
        WV8f = sg.tile([128, 2 * C], FP8)
        WO8f = sg.tile([128, 2 * C], FP8)
        WF8Lf = sg.tile([128, 2 * C], FP8)
        WF8Rf = sg.tile([128, 2 * C], FP8)
        # all lhsT layouts keep the ct slot-pair contiguous (dual-fp8
        # ldweights requires slot stride == inner size)
        WQ8 = WQ8f.rearrange("p (g s ct d) -> p g s ct d", g=2, s=2, ct=2)
        WK8 = WK8f.rearrange("p (g s ct d) -> p g s ct d", g=2, s=2, ct=2)
        WV8 = WV8f.rearrange("p (ct d) -> p ct d", ct=2)
        WO8 = WO8f.rearrange("p (mt ct d) -> p mt ct d", mt=2, ct=2)
        WF8L = WF8Lf.rearrange("p (s ct d) -> p s ct d", s=2, ct=2)
        WF8R = WF8Rf.rearrange("p (ct d) -> p ct d", ct=2)
        BQP = sg.tile([128, 4], F32)
        BKP = sg.tile([128, 4], F32)
        BVr = sg.tile([128, C], F32)
        BO = sg.tile([128, 2, 1], F32)
        BFP64 = sg.tile([128, 2], F32)
        BFr64 = sg.tile([128, C], F32)
        WRr = sg.tile([128, h2], F32)
        BRr = sg.tile([128, h2], F32)
        WCr = sg.tile([128, h2], F32)
        BCr = sg.tile([128, h2], F32)
        WGr = sg.tile([128, H], F32)
        BGr = sg.tile([128, H], F32)
        GT = sg.tile([128, 2, 1], F32)
        ONESZ = sg.tile([128, 4, 2, 128], FP8)    # dp lhsT, m<4
        ONESZ4 = sg.tile([128, 4, 2, 128], FP8)   # dp lhsT, m=4 (2 real keys)
        ONES2 = sg.tile([128, 2, 128], FP8)       # attn2 d2 lhsT

        for ct in range(2):
            for xc in range(4):
                nc.sync.dma_start(
                    out=X[:, ct, xc * 1024:(xc + 1) * 1024],
                    in_=x_d[ct * 128:(ct + 1) * 128, xc * 1024:(xc + 1) * 1024])
        nc.sync.dma_start(out=WQ8f[:, :], in_=wq8_d[:, :])
        nc.sync.dma_start(out=WK8f[:, :], in_=wk8_d[:, :])
        nc.sync.dma_start(out=WV8f[:, :], in_=wv8_d[:, :])
        nc.sync.dma_start(out=WO8f[:, :], in_=wo8_d[:, :])
        nc.sync.dma_start(out=WF8Lf[:, :], in_=wf8l_d[:, :])
        nc.sync.dma_start(out=WF8Rf[:, :], in_=wf8r_d[:, :])
        nc.sync.dma_start(out=BQP[:, :], in_=bqp_d[:, :])
        nc.sync.dma_start(out=BKP[:, :], in_=bkp_d[:, :])
        nc.sync.dma_start(out=BVr[:, :], in_=bvr_d[:, :])
        for mt in range(2):
            nc.sync.dma_start(out=BO[:, mt, :],
                              in_=bo_d[mt * 128:(mt + 1) * 128, :])
        nc.sync.dma_start(out=BFP64[:, :], in_=bfp64_d[:, :])
        nc.sync.dma_start(out=BFr64[:, :], in_=bfr64_d[:, :])
        for n, dst in [("wrow_rep", WRr), ("brow_rep", BRr),
                       ("wcol_rep", WCr), ("bcol_rep", BCr)]:
            nc.sync.dma_start(out=dst[:, :], in_=g_d[n][:, :])
        nc.sync.dma_start(out=WGr[:, :], in_=wgt_rep_d[:, :])
        nc.sync.dma_start(out=BGr[:, :], in_=bgt_rep_d[:, :])

        # one-time constant tiles: built on the (otherwise idle) Pool engine
        nc.gpsimd.memset(ONESZ[:, :, :, :], 0.0)
        nc.gpsimd.memset(ONESZ4[:, :, :, :], 0.0)
        nc.gpsimd.memset(ONES2[:, :, :], 1.0)
        for j in range(4):
            nc.gpsimd.memset(ONESZ[:, j, :, 32 * j:32 * j + 32], 1.0)
            nc.gpsimd.memset(ONESZ4[0:2, j, 0, 32 * j:32 * j + 32], 1.0)

        with (
            tc.tile_pool(name="ypool", bufs=2) as ypool,
            tc.tile_pool(name="y8pool", bufs=1) as y8pool,
            tc.tile_pool(name="qkpool", bufs=2) as qkpool,
            tc.tile_pool(name="v8zpool", bufs=2) as v8zpool,
            tc.tile_pool(name="atpool", bufs=2) as atpool,
            tc.tile_pool(name="aopool", bufs=1) as aopool,
            tc.tile_pool(name="scratch", bufs=2) as scratch,
            tc.tile_pool(name="gpool", bufs=1) as gpool,
            tc.tile_pool(name="ps_ring", bufs=1, space="PSUM") as ps_ring,
            tc.tile_pool(name="ps_acc", bufs=2, space="PSUM") as ps_acc,
        ):
            # pre-zero pad regions once per physical buffer (on Pool)
            for _ in range(2):
                At = atpool.tile([128, NSLOT, NH, QC], FP8, tag="At", name="At")
                nc.gpsimd.memset(At[:, 8:10, :, :], 0.0)
            for _ in range(2):
                V8Z = v8zpool.tile([128, 2, 4, NSLOT, 128], FP8, tag="V8Z",
                                   name="V8Z")
                nc.gpsimd.memset(V8Z[:, :, :, :, :], 0.0)
            V8 = v8zpool.tile([128, NSLOT, C], FP8, tag="V8", bufs=1,
                              name="V8")
            nc.gpsimd.memset(V8[:, :, :], 0.0)

            # global token: mean over all pixels, on ACT (accum_out) so the
            # startup DVE chain stays short
            GTP = scratch.tile([128, 2, 4], F32, tag="GTP", bufs=1, name="GTP")
            for ct in range(2):
                for xc in range(4):
                    scr = gpool.tile([128, 1024], BF16, tag="prod", name="scr")
                    nc.scalar.activation(out=scr[:, :],
                                         in_=X[:, ct, xc * 1024:(xc + 1) * 1024],
                                         func=AF.Copy,
                                         accum_out=GTP[:, ct, xc:xc + 1])
                nc.vector.reduce_sum(GT[:, ct, :], GTP[:, ct, :], AX.X)
                nc.vector.tensor_scalar_mul(GT[:, ct, :], GT[:, ct, :],
                                            1.0 / HW)

            quad_state = {}
            # Shared FIFO of deferred emissions. Every user of the shared
            # "acc" PSUM pool allocates its tile INSIDE its closure, so
            # allocation order == emission order and the 2-slot rotation
            # stays correctly nested.
            backlog = deque()

            def flush(k):
                for _ in range(min(k, len(backlog))):
                    backlog.popleft()()

            def acc_tile(name):
                return ps_acc.tile([128, 512], F32, tag="acc", name=name)

            def prep(q):
                """Emit Y/Y8 and queue Q/K/V projections for quad q."""
                Y = ypool.tile([128, 2, T], F32, tag="Y", name="Y")
                Y8 = y8pool.tile([128, 2, T], FP8, tag="Y8", name="Y8")
                # token-blocked fp8 Y for the V-proj lhsT (slot pair = ct must
                # be contiguous); slot 8 holds lt/gt + zero pad
                Y8V = y8pool.tile([128, NKT, 2, 128], FP8, tag="Y8V",
                                  name="Y8V")
                QDR = qkpool.tile([128, 2, 2, T], FP8, tag="QDR", name="QDR")
                # key-blocked K: [p, g, kt, ct-slot, 128]; kt=8 cols 2..128
                # are zero-padded fake keys (their exp=1 rows are masked by
                # V8Z/ONESZ4 zeros)
                KDR = qkpool.tile([128, 2, NKT, 2, 128], FP8, tag="KDR",
                                  name="KDR")
                V8Z = v8zpool.tile([128, 2, 4, NSLOT, 128], FP8, tag="V8Z",
                                   name="V8Zq")
                LTG = scratch.tile([128, 2, 2], F32, tag="LTG", name="LTG")
                quad_state[q] = dict(Y=Y, QDR=QDR, KDR=KDR, V8Z=V8Z, LTG=LTG)
                nc.vector.memset(Y8V[:, 8, :, 2:128], 0.0)
                nc.vector.memset(KDR[:, :, 8, :, 2:128], 0.0)
                r0, c0 = h2 * (q // 2), w2 * (q % 2)
                for ct in range(2):
                    xv = X[:, ct, :].rearrange("p (a b) -> p a b", a=H)[
                        :, r0:r0 + h2, c0:c0 + w2]
                    yq = Y[:, ct, 0:PIX].rearrange("p (a b) -> p a b", a=h2)
                    nc.vector.tensor_copy(yq, xv)
                    nc.vector.reduce_sum(LTG[:, ct, 0:1], Y[:, ct, 0:PIX], AX.X)
                    nc.vector.tensor_scalar_mul(LTG[:, ct, 0:1],
                                                LTG[:, ct, 0:1], 1.0 / PIX)
                    nc.vector.tensor_copy(LTG[:, ct, 1:2], GT[:, ct, :])
                    nc.vector.tensor_copy(Y[:, ct, PIX:T], LTG[:, ct, :])
                    nc.vector.tensor_copy(Y8[:, ct, :], Y[:, ct, :])
                    nc.vector.tensor_copy(
                        Y8V[:, 0:8, ct, :],
                        Y[:, ct, 0:PIX].rearrange("p (a b) -> p a b", b=128))
                    nc.vector.tensor_copy(Y8V[:, 8, ct, 0:2],
                                          Y[:, ct, PIX:T])
                    yield
                # Q projection (flat layout, rhs-only -> strided slots fine)
                for g in range(2):
                    for s in range(2):
                        for qs in range(0, T, QC):
                            def qp(g=g, s=s, qs=qs, Y8=Y8, QDR=QDR):
                                pq = acc_tile("pq")
                                nc.tensor.matmul(
                                    pq[:, 0:QC], WQ8[:, g, s, :, :],
                                    Y8[:, :, qs:qs + QC],
                                    start=True, stop=True, perf_mode=DR)
                                nc.vector.tensor_scalar_add(
                                    QDR[:, g, s, qs:qs + QC], pq[:, 0:QC],
                                    BQP[:, 2 * g + s:2 * g + s + 1])
                            backlog.append(qp)
                            yield
                # K projection into the key-blocked layout (256-token chunks)
                for g in range(2):
                    for s in range(2):
                        for c4 in range(4):
                            def kp(g=g, s=s, c4=c4, Y8=Y8, KDR=KDR):
                                pq = acc_tile("pk")
                                nc.tensor.matmul(
                                    pq[:, 0:256], WK8[:, g, s, :, :],
                                    Y8[:, :, 256 * c4:256 * c4 + 256],
                                    start=True, stop=True, perf_mode=DR)
                                nc.vector.tensor_scalar_add(
                                    KDR[:, g, 2 * c4:2 * c4 + 2, s, :],
                                    pq[:, 0:256].rearrange(
                                        "p (a b) -> p a b", b=128),
                                    BKP[:, 2 * g + s:2 * g + s + 1])
                            backlog.append(kp)
                            yield
                        def kp_tail(g=g, s=s, Y8=Y8, KDR=KDR):
                            pq = acc_tile("pkt")
                            nc.tensor.matmul(
                                pq[:, 0:2], WK8[:, g, s, :, :],
                                Y8[:, :, 1024:1026],
                                start=True, stop=True, perf_mode=DR)
                            nc.vector.tensor_scalar_add(
                                KDR[:, g, 8, s, 0:2], pq[:, 0:2],
                                BKP[:, 2 * g + s:2 * g + s + 1])
                        backlog.append(kp_tail)
                        yield
                for tt in range(NKT):
                    def vp(tt=tt, Y8V=Y8V):
                        n = 128 if tt < 8 else T - 8 * 128
                        pv = acc_tile("pv")
                        nc.tensor.matmul(pv[:, 0:C], Y8V[:, tt, :, :],
                                         WV8[:, :, :],
                                         start=True, stop=True, perf_mode=DR)
                        nc.vector.tensor_add(V8[0:n, tt, :], pv[0:n, 0:C],
                                             BVr[0:n, :])
                    backlog.append(vp)
                    yield

                def v8z_fill(V8Z=V8Z):
                    # scatter V8 into the zero-padded per-(g, j) lhsT layout
                    # on the Pool engine (keeps DVE free)
                    for g in range(2):
                        for j in range(4):
                            cs = 128 * g + 32 * j
                            nc.gpsimd.tensor_copy(
                                V8Z[:, g, j, :, 32 * j:32 * j + 32],
                                V8[:, :, cs:cs + 32])
                backlog.append(v8z_fill)
                yield

            def att(q):
                """Attention for quad q. Yields after each kt exp so the
                driver can interleave prep(q+1) work."""
                r0, c0 = h2 * (q // 2), w2 * (q % 2)
                st = quad_state[q]
                Y, QDR, KDR, V8Z, LTG = (st["Y"], st["QDR"], st["KDR"],
                                         st["V8Z"], st["LTG"])
                AT = aopool.tile([128, 2, T], FP8, tag="AT", name="AT")
                OT = aopool.tile([128, 2, T], F32, tag="OT", name="OT")

                # two independent 3-bank score buffers; windows alternate
                # between them (separate tiles so Tile never serializes a
                # write to one against an exp read of the other)
                sp2 = [ps_ring.tile([128, 3, 512], F32, tag=f"sp{i}",
                                    name=f"sp{i}") for i in range(2)]
                wcount = 0
                for qs in range(0, T, QC):
                    At = atpool.tile([128, NSLOT, NH, QC], FP8, tag="At",
                                     name="Atq")
                    for kt in range(NKT):
                        # 3 windows of heads (3, 3, 2); reuse is always two
                        # windows behind = double buffered
                        for h0, h1 in ((0, 3), (3, 6), (6, 8)):
                            sp = sp2[wcount % 2]
                            wcount += 1
                            for i, h in enumerate(range(h0, h1)):
                                g, j = h // 4, h % 4
                                nc.tensor.matmul(
                                    sp[:, i, 0:QC],
                                    KDR[32 * j:32 * j + 16, g, kt, :, :],
                                    QDR[32 * j:32 * j + 16, g, :, qs:qs + QC],
                                    start=True, stop=True, perf_mode=DR,
                                    tile_position=(32 * j, 0))
                            # deferred PE work goes AFTER this window's
                            # scores (so exp never waits on it) and BEFORE
                            # the next window's scores (small items only)
                            flush(3)
                            nc.scalar.activation(
                                out=At[:, kt, h0:h1, :],
                                in_=sp[:, 0:h1 - h0, 0:QC],
                                func=AF.Exp)
                            yield

                    for g in range(2):
                        # queue AV + dp + normalize for this (g, qs); the
                        # avp/dp banks are allocated at flush time (m == 0)
                        # to keep the acc-pool rotation FIFO
                        hold = {}

                        def mk_av(m, j, g=g, At=At, hold=hold, V8Z=V8Z):
                            def go():
                                if m == 0 and j == 0:
                                    hold["avp"] = acc_tile("avp")
                                    hold["dp"] = acc_tile("dp")
                                avp, dp = hold["avp"], hold["dp"]
                                nc.tensor.matmul(
                                    avp[:, 0:QC],
                                    V8Z[:, g, j, 2 * m:2 * m + 2, :],
                                    At[:, 2 * m:2 * m + 2, 4 * g + j, :],
                                    start=(m == 0 and j == 0),
                                    stop=(m == 4 and j == 3),
                                    perf_mode=DR)
                                oz = ONESZ if m < 4 else ONESZ4
                                nc.tensor.matmul(
                                    dp[:, 0:QC],
                                    oz[:, j, :, :],
                                    At[:, 2 * m:2 * m + 2, 4 * g + j, :],
                                    start=(m == 0 and j == 0),
                                    stop=(m == 4 and j == 3),
                                    perf_mode=DR)
                            return go

                        for m in range(NKG):
                            for j in range(4):
                                backlog.append(mk_av(m, j))

                        def norm(g=g, qs=qs, hold=hold, AT=AT):
                            dr = scratch.tile([128, QC], F32, tag="dr",
                                              name="dr")
                            nc.vector.reciprocal(dr[:, :], hold["dp"][:, 0:QC])
                            nc.vector.tensor_mul(AT[:, g, qs:qs + QC],
                                                 hold["avp"][:, 0:QC],
                                                 dr[:, :])
                        backlog.append(norm)

                        if g == 1:
                            def oproj(qs=qs, AT=AT, OT=OT, Y=Y):
                                for mt in range(2):
                                    po = acc_tile("po")
                                    nc.tensor.matmul(
                                        po[:, 0:QC],
                                        WO8[:, mt, :, :],
                                        AT[:, :, qs:qs + QC],
                                        start=True, stop=True, perf_mode=DR)
                                    nc.vector.scalar_tensor_tensor(
                                        OT[:, mt, qs:qs + QC], po[:, 0:QC],
                                        BO[:, mt, :], Y[:, mt, qs:qs + QC],
                                        op0=ALU.add, op1=ALU.add)
                            backlog.append(oproj)
                while backlog:
                    backlog.popleft()()
                    yield

                # gating
                for ct in range(2):
                    ltp = OT[:, ct, PIX:PIX + 1]
                    gtp = OT[:, ct, PIX + 1:T]
                    row = gpool.tile([128, h2], F32, tag="row", name="row")
                    col = gpool.tile([128, h2], F32, tag="col", name="col")
                    nc.vector.scalar_tensor_tensor(row[:, :], WRr[:, :], ltp,
                                                   BRr[:, :], op0=ALU.mult,
                                                   op1=ALU.add)
                    nc.vector.scalar_tensor_tensor(col[:, :], WCr[:, :], ltp,
                                                   BCr[:, :], op0=ALU.mult,
                                                   op1=ALU.add)
                    prod = gpool.tile([128, h2, w2], BF16, tag="prod",
                                      name="prod")
                    nc.vector.tensor_mul(
                        prod[:, :, :],
                        row[:, :, None].broadcast_to([128, h2, w2]),
                        col[:, None, :].broadcast_to([128, h2, w2]))
                    eg = gpool.tile([128, h2, w2], BF16, tag="eg", name="eg")
                    nc.scalar.activation(out=eg[:, :, :], in_=prod[:, :, :],
                                         func=AF.Exp)
                    # sigmoid(z) = e/(1+e): stays on the Exp ACT table
                    e1 = gpool.tile([128, h2, w2], BF16, tag="prod", name="e1")
                    nc.vector.tensor_scalar_add(e1[:, :, :], eg[:, :, :], 1.0)
                    with nc.allow_low_precision(reason="sigmoid gate in bf16"):
                        nc.vector.reciprocal(e1[:, :, :], e1[:, :, :])
                    nc.vector.tensor_mul(eg[:, :, :], eg[:, :, :], e1[:, :, :])
                    fv = FW[:, ct, :].rearrange("p (a b) -> p a b", a=H)[
                        :, r0:r0 + h2, c0:c0 + w2]
                    xp = OT[:, ct, 0:PIX].rearrange("p (a b) -> p a b", a=h2)
                    nc.vector.tensor_mul(fv, xp, eg[:, :, :])
                    rp_v = RP[:, ct, :].rearrange("p (a b) -> p a b", b=4)[
                        :, :, q]
                    nc.vector.scalar_tensor_tensor(rp_v, WGr[:, :], gtp,
                                                   BGr[:, :], op0=ALU.mult,
                                                   op1=ALU.add)
                    yield

            def drain(gen):
                if gen is None:
                    return
                for _ in gen:
                    pass

            drain(prep(0))
            # prep(0)'s projection closures must be emitted before att(0)
            # emits scores that read QDR/KDR (no writer yet = no dependency)
            while backlog:
                backlog.popleft()()
            for q in range(4):
                a = att(q)
                p = prep(q + 1) if q < 3 else None
                while True:
                    try:
                        next(a)
                    except StopIteration:
                        break
                    if p is not None:
                        for _ in range(2):
                            try:
                                next(p)
                            except StopIteration:
                                p = None
                                break
                drain(p)

        _emit_attn2(nc, tc, FW, RP, WF8L, WF8R, BFP64, BFr64, ONES2, o_d)


def _emit_attn2(nc, tc, FW, RP, WF8L, WF8R, BFP64, BFr64, ONES2, o_d):
    with (
        tc.tile_pool(name="a2pool", bufs=1) as a2pool,
        tc.tile_pool(name="opool", bufs=2) as opool,
        tc.tile_pool(name="ps2_s", bufs=2, space="PSUM") as ps2_s,
        tc.tile_pool(name="ps2_acc", bufs=2, space="PSUM") as ps2_acc,
    ):
        FW8 = a2pool.tile([128, 2, HW], FP8, tag="FW8", name="FW8")
        RP8R = a2pool.tile([128, 2, 4 * H], FP8, tag="RP8R", name="RP8R")
        RP8L = a2pool.tile([128, 2, 2, 128], FP8, tag="RP8L", name="RP8L")
        K2T8 = a2pool.tile([128, 2, 2, 128], FP8, tag="K2T8", name="K2T8")
        K2K8 = a2pool.tile([128, 2, 2, 128], FP8, tag="K2K8", name="K2K8")
        A28 = a2pool.tile([128, 2, HW], FP8, tag="A28", name="A28")

        for ct in range(2):
            nc.vector.tensor_copy(FW8[:, ct, :], FW[:, ct, :])
            nc.vector.tensor_scalar_mul(RP8R[:, ct, :], RP[:, ct, :], K2SC)
            nc.vector.tensor_scalar_mul(
                RP8L[:, :, ct, :],
                RP[:, ct, :].rearrange("p (a b) -> p a b", b=128), K2SC)

        # K2T8[p, kt, s, key] = 64*K2T[cout=128s+p, key=128kt+key]
        # K2K8[p, mt, s, c] = 64*K2K[key=128s+p, cout=128mt+c]
        for s in range(2):
            pk = ps2_acc.tile([128, 512], F32, tag="acc2", name="pk")
            nc.tensor.matmul(pk[:, 0:C], WF8L[:, s, :, :],
                             RP8R[:, :, :], start=True, stop=True,
                             perf_mode=DR)
            nc.vector.tensor_scalar_add(
                K2T8[:, :, s, :],
                pk[:, 0:C].rearrange("p (a b) -> p a b", b=128),
                BFP64[:, s:s + 1])
        for s in range(2):
            pk = ps2_acc.tile([128, 512], F32, tag="acc2", name="pk2")
            nc.tensor.matmul(pk[:, 0:C], RP8L[:, s, :, :],
                             WF8R[:, :, :], start=True, stop=True,
                             perf_mode=DR)
            nc.vector.tensor_add(
                K2K8[:, :, s, :],
                pk[:, 0:C].rearrange("p (a b) -> p a b", b=128),
                BFr64[:, :].rearrange("p (a b) -> p a b", b=128))

        a2scale = 1.0 / (math.sqrt(C) * K2SC)

        def a2_scores(qb):
            s2 = ps2_s.tile([128, 2, 512], F32, tag="s2", name="s2")
            for kt in range(2):
                nc.tensor.matmul(s2[:, kt, :],
                                 K2T8[:, kt, :, :],
                                 FW8[:, :, qb:qb + 512],
                                 start=True, stop=True, perf_mode=DR)
            nc.scalar.activation(out=A28[:, :, qb:qb + 512], in_=s2[:, :, :],
                                 func=AF.Exp, scale=a2scale)

        def a2_out(qb):
            d2 = ps2_acc.tile([128, 512], F32, tag="acc2", name="d2")
            nc.tensor.matmul(d2[:, :], ONES2[:, :, :], A28[:, :, qb:qb + 512],
                             start=True, stop=True, perf_mode=DR)
            dr2 = opool.tile([128, 512], F32, tag="dr2", name="dr2")
            nc.vector.reciprocal(dr2[:, :], d2[:, :])
            for mt in range(2):
                f2 = ps2_acc.tile([128, 512], F32, tag="acc2", name="f2")
                nc.tensor.matmul(f2[:, :],
                                 K2K8[:, mt, :, :],
                                 A28[:, :, qb:qb + 512],
                                 start=True, stop=True, perf_mode=DR)
                tmp = opool.tile([128, 512], F32, tag="tmp", name="tmp")
                nc.vector.scalar_tensor_tensor(tmp[:, :], f2[:, :],
                                               1.0 / K2SC, dr2[:, :],
                                               op0=ALU.mult, op1=ALU.mult)
                outc = opool.tile([128, 512], F32, tag="outc", name="outc")
                nc.vector.tensor_add(outc[:, :], tmp[:, :],
                                     FW[:, mt, qb:qb + 512])
                nc.sync.dma_start(
                    out=o_d[mt * 128:(mt + 1) * 128, qb:qb + 512],
                    in_=outc[:, :])

        a2_scores(0)
        a2_scores(512)
        for qb in range(0, HW, 512):
            a2_out(qb)
            if qb + 1024 < HW:
                a2_scores(qb + 1024)


_NC_CACHE = None


def _get_nc():
    global _NC_CACHE
    if _NC_CACHE is None:
        _NC_CACHE = _build()
    return _NC_CACHE


def _prep_inputs(inputs):
    f = np.float32
    f8 = ml_dtypes.float8_e4m3fn
    s = 1.0 / math.sqrt(HD)
    x = np.asarray(inputs["x"], f).reshape(B, C, HW)

    # channel permutation for the Q/K DR-pair layout:
    # tile-group g, partition p = 32j + k (k<16 used, k>=16 duplicates), slot
    # s2 -> channel 32*(4g+j) + (p%32)%16 + 16*s2
    p_arr = np.arange(128)
    j_arr = p_arr // 32
    k_arr = (p_arr % 32) % 16
    cperm = np.zeros((2, 2, 128), np.int64)   # [g, s2, p] -> channel
    for g in range(2):
        for s2 in range(2):
            cperm[g, s2] = 32 * (4 * g + j_arr) + k_arr + 16 * s2

    def qk_w(wmat, scale):
        # wT[cin, cout] = wmat.T * scale; out [128, g, s2, ct, 128] fp8
        # (ct slot-pair contiguous for the dual-fp8 ldweights)
        wT = (np.asarray(wmat, f).T * scale)
        out = np.zeros((128, 2, 2, 2, 128), f)
        for g in range(2):
            for s2 in range(2):
                for ct in range(2):
                    out[:, g, s2, ct, :] = wT[128 * ct:128 * (ct + 1),
                                              cperm[g, s2]]
        return out.reshape(128, -1).astype(f8)

    def qk_b(bvec, scale):
        b = np.asarray(bvec, f) * scale
        out = np.zeros((128, 2, 2), f)
        for g in range(2):
            for s2 in range(2):
                out[:, g, s2] = b[cperm[g, s2]]
        return out.reshape(128, 4).copy()

    def dr_w(wmat):
        # moving-side layout [cin-pair partitions, (ct, cout)] fp8
        wT = np.asarray(wmat, f).T  # [cin, cout]
        return np.ascontiguousarray(
            wT.reshape(2, 128, C).transpose(1, 0, 2).reshape(128, 2 * C)
        ).astype(f8)

    def dr_wl(wmat):
        # stationary-side layout [cin-pair partitions, (cout-tile, ct, d)]
        wT = np.asarray(wmat, f).T  # [cin, cout]
        out = np.zeros((128, 2, 2, 128), f)
        for mt in range(2):
            for ct in range(2):
                out[:, mt, ct, :] = wT[128 * ct:128 * (ct + 1),
                                       128 * mt:128 * (mt + 1)]
        return out.reshape(128, -1).astype(f8)

    base = {
        "wq8": qk_w(inputs["Wq"], s),
        "wk8": qk_w(inputs["Wk"], 1.0),
        "wv8": dr_w(inputs["Wv"]),
        "wo8": dr_wl(inputs["Wo"]),
        "wf8l": dr_wl(inputs["Wfuse"]),
        "wf8r": dr_w(inputs["Wfuse"]),
        "bqp": qk_b(inputs["bq"], s),
        "bkp": qk_b(inputs["bk"], 1.0),
        "bvr": np.broadcast_to(np.asarray(inputs["bv"], f), (128, C)).copy(),
        "bo": np.asarray(inputs["bo"], f).reshape(C, 1).copy(),
        "bfp64": np.asarray(inputs["bfuse"], f).reshape(2, 128).T.copy() * K2SC,
        "bfr64": np.broadcast_to(np.asarray(inputs["bfuse"], f) * K2SC,
                                 (128, C)).copy(),
        "wrow_rep": np.broadcast_to(np.asarray(inputs["w_row"], f),
                                    (128, h2)).copy(),
        "brow_rep": np.broadcast_to(np.asarray(inputs["b_row"], f),
                                    (128, h2)).copy(),
        "wcol_rep": np.broadcast_to(np.asarray(inputs["w_col"], f),
                                    (128, h2)).copy(),
        "bcol_rep": np.broadcast_to(np.asarray(inputs["b_col"], f),
                                    (128, h2)).copy(),
        "wgt_rep": np.broadcast_to(np.asarray(inputs["w_gt"], f),
                                   (128, H)).copy(),
        "bgt_rep": np.broadcast_to(np.asarray(inputs["b_gt"], f),
                                   (128, H)).copy(),
    }
    return [dict(base, x=np.ascontiguousarray(x[b])) for b in range(B)]


def _run(inputs, **kwargs):
    nc = _get_nc()
    in_maps = _prep_inputs(inputs)
    return run_bass_kernel_spmd(nc, in_maps, core_ids=list(range(B)), **kwargs)


def kernel(**inputs) -> np.ndarray:
    res = _run(inputs)
    out = np.stack([r["o"] for r in res.results], axis=0)
    return out.reshape(B, C, H, W)
